# revision 13
# baseline (speedup 1.0000x reference)
"""Trainium2 Bass kernel for nn_Decoder_1D_Matryoshka (12-layer masked decoder).

v2: exploits the Matryoshka sparsity. Rows >= 256+Ni are dead (never attended
by any row that reaches the output), so each sample only needs T_i = 256+Ni
tokens. Samples are split into two shape classes (TA = 8 largest, TB = rest),
one of each per core; per-sample masks/zero-padding carry correctness so the
SPMD program is identical across cores. Attention exploits the block mask
structure: mask-token keys are only attended by mask-token queries, latent
keys only by causally-later latent queries + all mask queries. The Ni cutoff
is a per-key-partition exp bias; only the 128x128 causal diagonal needs a
tensor mask add (static triangle). Softmax denominators come free from the
AV matmul via an interleaved ones column in V. Weight DMAs are hoisted so
each layer's weights stream once (not once per sample).

Residual stream kept transposed (features on partitions, tokens free) as in
v1; attention uses transposed scores S^T = K Q^T.
"""

import os
import numpy as np
import ml_dtypes

B, NLAT, DIN = 16, 256, 32
D, H, NL, VAE = 1024, 16, 12, 16
M = 256
DH = 64
NCORES = 8
BPC = 2
P = 128
KT = D // P       # 8 k-bands over D
FF = 4 * D
NL_OVR = int(os.environ.get("BASSK_NL", NL))

BF16 = ml_dtypes.bfloat16
FP8NP = ml_dtypes.float8_e4m3
FP8 = os.environ.get("BASSK_FP8", "0") == "1"
WS = 64.0         # fp8 weight scale

# param-tile column layout (one [128, 104] f32 DMA per layer)
PC_BQKV = 0       # 24 cols: qkv bias, n-tile t at col t (q cols pre-scaled)
PC_BPROJ = 24     # 8
PC_B1 = 32        # 32
PC_B2 = 64        # 8
PC_G1 = 72        # 8
PC_BL1 = 80       # 8
PC_G2 = 88        # 8
PC_BL2 = 96       # 8
PCOLS = 104


def _layernorm_np(x, g, b, eps=1e-5):
    mu = x.mean(-1, keepdims=True)
    var = ((x - mu) ** 2).mean(-1, keepdims=True)
    return (x - mu) / np.sqrt(var + eps) * g + b


def _classes(ni):
    T = 256 + np.asarray(ni).astype(np.int64)
    Tp = ((T + 127) // 128) * 128
    order = np.argsort(-Tp, kind="stable")
    TA = int(Tp[order[0]])
    TB = int(Tp[order[NCORES]])
    return order, TA, TB


def _host_prep(inputs):
    """Returns (per_core_in_maps, order, TA, TB)."""
    f32 = np.float32
    ni = np.asarray(inputs["num_activated"]).astype(np.int64)
    order, TA, TB = _classes(ni)
    CT = TA + TB

    lat = inputs["latents"].astype(f32)
    x_lat = lat.reshape(B * NLAT, DIN) @ inputs["input_w"].astype(f32)
    x_lat = x_lat.reshape(B, NLAT, D) + inputs["input_b"][None, None, :]
    x_lat = x_lat + inputs["latents_pos_embed"][None, :, :]
    mt = inputs["mask_tokens"].reshape(1, 1, D) + inputs["pos_embed_full"]
    mt = np.broadcast_to(mt, (B, M, D))
    x = np.concatenate([mt, x_lat], axis=1)                       # (B, L, D)
    x = _layernorm_np(x, inputs["ln_pre_g"], inputs["ln_pre_b"]).astype(f32)

    # static causal triangle for the diagonal 128x128 latent blocks
    # tri[key, query] = -30 where key > query
    rr = np.arange(P)
    tri = np.where(rr[:, None] <= rr[None, :], f32(0.0), f32(-30.0))

    # ---- static (identical on every core) weights, repacked ----
    scale = np.float32(DH ** -0.5)
    wqkv = inputs["qkv_w"].astype(f32).copy()                      # (NL,D,3D)
    wqkv[:, :, :D] *= scale
    bqkv = inputs["qkv_b"].astype(f32).copy()                      # (NL,3D)
    bqkv[:, :D] *= scale

    params = np.zeros((NL, P, PCOLS), f32)
    params[:, :, PC_BQKV:PC_BQKV + 24] = bqkv.reshape(NL, 24, P).transpose(0, 2, 1)
    params[:, :, PC_BPROJ:PC_BPROJ + 8] = inputs["proj_b"].reshape(NL, 8, P).transpose(0, 2, 1)
    params[:, :, PC_B1:PC_B1 + 32] = inputs["fc1_b"].reshape(NL, 32, P).transpose(0, 2, 1)
    params[:, :, PC_B2:PC_B2 + 8] = inputs["fc2_b"].reshape(NL, 8, P).transpose(0, 2, 1)
    params[:, :, PC_G1:PC_G1 + 8] = inputs["ln1_g"].reshape(NL, 8, P).transpose(0, 2, 1)
    params[:, :, PC_BL1:PC_BL1 + 8] = inputs["ln1_b"].reshape(NL, 8, P).transpose(0, 2, 1)
    params[:, :, PC_G2:PC_G2 + 8] = inputs["ln2_g"].reshape(NL, 8, P).transpose(0, 2, 1)
    params[:, :, PC_BL2:PC_BL2 + 8] = inputs["ln2_b"].reshape(NL, 8, P).transpose(0, 2, 1)

    # v-bias rows, seeded into psum via K=1 ones-matmul: (NL, 1, D)
    rowparams = bqkv[:, 2 * D:3 * D].reshape(NL, 1, D).astype(f32)

    postparams = np.zeros((P, 16), f32)
    postparams[:, 0:8] = inputs["ln_post_g"].reshape(8, P).T
    postparams[:, 8:16] = inputs["ln_post_b"].reshape(8, P).T

    if FP8:
        def q8(w):
            return np.ascontiguousarray(
                np.clip(w.astype(f32) * WS, -240, 240).astype(FP8NP))
        wmain = {
            "wqkv": q8(wqkv),
            "wproj": q8(inputs["proj_w"]),
            "w1": q8(inputs["fc1_w"]),
            "w2": q8(inputs["fc2_w"]),
        }
        rowparams = rowparams * np.float32(WS)
    else:
        wmain = {
            "wqkv": np.ascontiguousarray(wqkv.astype(BF16)),
            "wproj": np.ascontiguousarray(inputs["proj_w"].astype(BF16)),
            "w1": np.ascontiguousarray(inputs["fc1_w"].astype(BF16)),
            "w2": np.ascontiguousarray(inputs["fc2_w"].astype(BF16)),
        }
    static = {
        **wmain,
        "params": params,
        "rowparams": np.ascontiguousarray(rowparams.astype(BF16)),
        "postparams": postparams,
        "tri": np.ascontiguousarray(tri.astype(BF16)),
        "wout": np.ascontiguousarray(
            inputs["out_w"].astype(BF16).reshape(KT, P, VAE)),
        "bout": inputs["out_b"].astype(f32).reshape(VAE, 1),
    }

    in_maps = []
    for c in range(NCORES):
        sA, sB = int(order[c]), int(order[NCORES + c])
        x0T = np.zeros((D, CT), f32)
        x0T[:, :256 + ni[sA]] = x[sA, :256 + ni[sA]].T
        x0T[:, TA:TA + 256 + ni[sB]] = x[sB, :256 + ni[sB]].T
        # per-latent-tile cutoff bias: cols 0..1 sample A tiles, 2..3 sample B
        latcut = np.full((P, 4), f32(-30.0))
        for j in range((TA - 256) // P):
            latcut[:, j] = np.where(j * P + rr < ni[sA], f32(0.0), f32(-30.0))
        for j in range((TB - 256) // P):
            latcut[:, 2 + j] = np.where(j * P + rr < ni[sB], f32(0.0), f32(-30.0))
        m = {"x0T": x0T, "latcut": latcut}
        m.update(static)
        in_maps.append(m)
    return in_maps, order, TA, TB


_PROG = None
_PROG_KEY = None


def _build_bass(TA, TB, zb):
    import concourse.bass as bass
    import concourse.bacc as bacc
    import concourse.tile as tile
    import concourse.mybir as mybir
    from contextlib import ExitStack

    f32 = mybir.dt.float32
    bf16 = mybir.dt.bfloat16
    f8 = mybir.dt.float8e4
    wdt = f8 if FP8 else bf16
    DR = mybir.MatmulPerfMode.DoubleRow if FP8 else None
    DQ = 1.0 / WS
    AF = mybir.ActivationFunctionType
    OP = mybir.AluOpType
    NLn = NL_OVR
    CT = TA + TB
    JA = TA // P          # key tiles sample A (4 when TA=512)
    JB = TB // P
    SOFF = (0, TA)
    STOK = (TA, TB)
    SJ = (JA, JB)
    SLC = (0, 2)          # latcut col base per sample

    nc = bacc.Bacc(None, target_bir_lowering=False, debug=False)
    dp = nc.declare_dram_parameter
    x0T = dp("x0T", [D, CT], f32, isOutput=False)
    latcut_d = dp("latcut", [P, 4], f32, isOutput=False)
    tri_d = dp("tri", [P, P], bf16, isOutput=False)
    wqkv = dp("wqkv", [NL, D, 3 * D], wdt, isOutput=False)
    wproj = dp("wproj", [NL, D, D], wdt, isOutput=False)
    w1 = dp("w1", [NL, D, FF], wdt, isOutput=False)
    w2 = dp("w2", [NL, FF, D], wdt, isOutput=False)
    params = dp("params", [NL, P, PCOLS], f32, isOutput=False)
    rowparams = dp("rowparams", [NL, 1, D], bf16, isOutput=False)
    postparams = dp("postparams", [P, 16], f32, isOutput=False)
    wout_d = dp("wout", [KT, P, VAE], bf16, isOutput=False)
    bout_d = dp("bout", [VAE, 1], f32, isOutput=False)
    out_d = dp("out", [BPC, M, VAE], f32, isOutput=True)

    with ExitStack() as ctx:
        tc = ctx.enter_context(tile.TileContext(nc))
        # ---- pools ----
        persist = ctx.enter_context(tc.tile_pool(name="persist", bufs=1))
        wpool = ctx.enter_context(tc.tile_pool(name="wslab", bufs=16))
        w2pool = ctx.enter_context(tc.tile_pool(name="w2slab", bufs=18))
        hpool = ctx.enter_context(tc.tile_pool(name="h", bufs=1))
        qkpool = ctx.enter_context(tc.tile_pool(name="qk", bufs=1))
        apool = ctx.enter_context(tc.tile_pool(name="attnT", bufs=1))
        gpool = ctx.enter_context(tc.tile_pool(name="g", bufs=1))
        epool = ctx.enter_context(tc.tile_pool(name="expS", bufs=2))
        tpool = ctx.enter_context(tc.tile_pool(name="scratch", bufs=2))
        spool = ctx.enter_context(tc.tile_pool(name="stats", bufs=5))
        rpool = ctx.enter_context(tc.tile_pool(name="rsb", bufs=2))
        bpool = ctx.enter_context(tc.tile_pool(name="lnb", bufs=1))
        ppool = ctx.enter_context(tc.tile_pool(name="lparams", bufs=2))
        ypool = ctx.enter_context(tc.tile_pool(name="yout", bufs=1))
        psp = ctx.enter_context(tc.tile_pool(name="ps", bufs=1, space="PSUM"))

        # ---- persistent tiles ----
        xt = [persist.tile([P, CT], f32, tag=f"x{k}", name=f"x{k}") for k in range(KT)]
        ones_f = persist.tile([P, 1], f32, tag="ones_f", name="ones_f")
        ones_b = persist.tile([P, 1], bf16, tag="ones_b", name="ones_b")
        ones_row = persist.tile([1, P], bf16, tag="ones_row", name="ones_row")
        trit = persist.tile([P, P], bf16, tag="tri", name="tri")
        latc = persist.tile([P, 4], f32, tag="latc", name="latc")
        # V with interleaved ones column per head: [P, J, 16 heads, 65]
        vtt = [persist.tile([P, SJ[s], H, DH + 1], bf16, tag=f"vt{s}", name=f"vt{s}")
               for s in range(BPC)]
        woutt = persist.tile([P, KT * VAE], bf16, tag="wo", name="wo")
        boutt = persist.tile([VAE, 1], f32, tag="bout", name="bout")
        postt = persist.tile([P, 16], f32, tag="post", name="post")
        epst = persist.tile([1, 1], f32, tag="eps", name="eps")
        nc.vector.memset(epst, 1e-5)
        zerot = persist.tile([P, 1], f32, tag="zerot", name="zerot")
        nc.vector.memset(zerot, 0.0)

        for k in range(KT):
            nc.sync.dma_start(out=xt[k], in_=x0T[k * P:(k + 1) * P, :])
            nc.sync.dma_start(out=woutt[:, k * VAE:(k + 1) * VAE], in_=wout_d[k])
        nc.vector.memset(ones_f, 1.0)
        nc.vector.memset(ones_b, 1.0)
        nc.vector.memset(ones_row, 1.0)
        for s in range(BPC):
            nc.vector.memset(vtt[s], 1.0)
        nc.sync.dma_start(out=trit, in_=tri_d[:, :])
        nc.sync.dma_start(out=latc, in_=latcut_d[:, :])
        nc.sync.dma_start(out=boutt, in_=bout_d[:, :])
        nc.sync.dma_start(out=postt, in_=postparams[:, :])

        def emit_ln(src_off, width, g_ap, b_ap, hsl, h_off, xq=None):
            """LN over features (partitions) of x[:, src_off:src_off+width] ->
            hsl(k, h_off, h_off+width). xq: optional precomputed full-width
            (xb, xsq) bf16 tiles."""
            psx = psp.tile([1, 512], f32, tag="row", bufs=2, name="psx")
            psxx = psp.tile([1, 512], f32, tag="row", bufs=2, name="psxx")
            for k in range(KT):
                xs = xt[k][:, src_off:src_off + width]
                if xq is None:
                    xsq = tpool.tile([P, 512], bf16, tag="xsq", name="xsq")
                    xb = tpool.tile([P, 512], bf16, tag="xb", name="xb")
                    nc.scalar.activation(xsq[:, :width], xs, AF.Square)
                    nc.vector.tensor_copy(out=xb[:, :width], in_=xs)
                    xb_ap = xb[:, :width]
                    xsq_ap = xsq[:, :width]
                else:
                    xb_ap = xq[0][k][:, src_off:src_off + width]
                    xsq_ap = xq[1][k][:, src_off:src_off + width]
                nc.tensor.matmul(psx[:, :width], ones_b, xb_ap,
                                 start=(k == 0), stop=(k == KT - 1))
                nc.tensor.matmul(psxx[:, :width], ones_b, xsq_ap,
                                 start=(k == 0), stop=(k == KT - 1))
            mu = spool.tile([1, 512], f32, tag="stat", name="mu")
            e2 = spool.tile([1, 512], f32, tag="stat", name="e2")
            rstd = spool.tile([1, 512], f32, tag="stat", name="rstd")
            nc.vector.tensor_scalar_mul(mu[:, :width], psx[:, :width], 1.0 / D)
            nc.vector.tensor_scalar_mul(e2[:, :width], psxx[:, :width], 1.0 / D)
            musq = spool.tile([1, 512], f32, tag="stat", name="musq")
            nc.vector.tensor_mul(musq[:, :width], mu[:, :width], mu[:, :width])
            nc.vector.tensor_sub(e2[:, :width], e2[:, :width], musq[:, :width])
            nc.scalar.activation(e2[:, :width], e2[:, :width], AF.Sqrt,
                                 bias=epst)
            nc.vector.reciprocal(rstd[:, :width], e2[:, :width])
            pmu = bpool.tile([P, 512], f32, tag="pmu", name="pmu")
            prs = bpool.tile([P, 512], f32, tag="prs", name="prs")
            nc.gpsimd.partition_broadcast(pmu[:, :width], mu[:, :width])
            nc.gpsimd.partition_broadcast(prs[:, :width], rstd[:, :width])
            for k in range(KT):
                xs = xt[k][:, src_off:src_off + width]
                t = tpool.tile([P, 512], f32, tag="lnt", name="lnt")
                nc.vector.tensor_sub(t[:, :width], xs, pmu[:, :width])
                nc.vector.tensor_mul(t[:, :width], t[:, :width], prs[:, :width])
                nc.vector.tensor_scalar(
                    out=hsl(k, h_off, h_off + width), in0=t[:, :width],
                    scalar1=g_ap[k], scalar2=b_ap[k],
                    op0=OP.mult, op1=OP.add)

        for l in range(NLn):
            pt = ppool.tile([P, PCOLS], f32, tag="pt", name="pt")
            rowp = ppool.tile([1, D], bf16, tag="rowp", name="rowp")
            nc.sync.dma_start(out=pt, in_=params[l])
            nc.sync.dma_start(out=rowp, in_=rowparams[l])
            g1 = [pt[:, PC_G1 + k:PC_G1 + k + 1] for k in range(KT)]
            bl1 = [pt[:, PC_BL1 + k:PC_BL1 + k + 1] for k in range(KT)]
            g2 = [pt[:, PC_G2 + k:PC_G2 + k + 1] for k in range(KT)]
            bl2 = [pt[:, PC_BL2 + k:PC_BL2 + k + 1] for k in range(KT)]

            # ---------------- LN1 -> h (both samples) ----------------
            if FP8:
                hp = [hpool.tile([P, 2, CT], f8, tag=f"h{i}", name=f"h{i}")
                      for i in range(4)]
                hsl = lambda k, lo, hi: hp[k // 2][:, k % 2, lo:hi]
            else:
                htiles = [hpool.tile([P, CT], bf16, tag=f"h{k}", name=f"h{k}") for k in range(KT)]
                hsl = lambda k, lo, hi: htiles[k][:, lo:hi]
            for s in range(BPC):
                emit_ln(SOFF[s], STOK[s], g1, bl1, hsl, SOFF[s])

            def load_wpair(src_d, cl, ch):
                wts = [wpool.tile([P, 2, ch - cl], f8, tag="wslab",
                                  name="wslab") for _ in range(4)]
                for pr in range(4):
                    for hf in range(2):
                        nc.sync.dma_start(
                            out=wts[pr][:, hf, :],
                            in_=src_d[l, (2 * pr + hf) * P:(2 * pr + hf + 1) * P,
                                      cl:ch])
                return wts

            def chainA(ps_ap, wts, col, off, T, seed=None):
                if seed is not None:
                    nc.tensor.matmul(ps_ap, ones_row, seed,
                                     start=True, stop=False)
                if FP8:
                    for pr in range(4):
                        nc.tensor.matmul(
                            ps_ap, wts[pr][:, :, col:col + P],
                            hp[pr][:, :, off:off + T],
                            start=(pr == 0 and seed is None), stop=(pr == 3),
                            perf_mode=DR)
                else:
                    for k in range(KT):
                        nc.tensor.matmul(
                            ps_ap, wts[k][:, col:col + P],
                            htiles[k][:, off:off + T],
                            start=(k == 0 and seed is None), stop=(k == KT - 1))

            # ---------------- QKV (weights hoisted over samples) -----
            qt = [qkpool.tile([P, CT], bf16, tag=f"q{n}", name=f"q{n}") for n in range(KT)]
            kt_ = [qkpool.tile([P, CT], bf16, tag=f"k{n}", name=f"k{n}") for n in range(KT)]
            for slab in range(4):          # cols 0..2048 of wqkv: q then k
                if FP8:
                    wts = load_wpair(wqkv, slab * 512, (slab + 1) * 512)
                else:
                    wts = [wpool.tile([P, 512], bf16, tag="wslab", name="wslab")
                           for _ in range(KT)]
                    for k in range(KT):
                        nc.sync.dma_start(
                            out=wts[k],
                            in_=wqkv[l, k * P:(k + 1) * P,
                                     slab * 512:(slab + 1) * 512])
                for ni_ in range(4):
                    nt = slab * 4 + ni_
                    dst = qt[nt] if nt < KT else kt_[nt - KT]
                    for s in range(BPC):
                        off, T = SOFF[s], STOK[s]
                        ps = psp.tile([P, 512], f32, tag="mm", bufs=3, name="psmm")
                        chainA(ps[:, :T], wts, ni_ * P, off, T)
                        if FP8:
                            nc.vector.tensor_scalar(
                                out=dst[:, off:off + T], in0=ps[:, :T],
                                scalar1=DQ,
                                scalar2=pt[:, PC_BQKV + nt:PC_BQKV + nt + 1],
                                op0=OP.mult, op1=OP.add)
                        else:
                            nc.vector.tensor_scalar_add(
                                dst[:, off:off + T], ps[:, :T],
                                pt[:, PC_BQKV + nt:PC_BQKV + nt + 1])
            # v: scheme B (h-stationary) -> [tokens, head, 64] w/ ones col
            for slab in range(4, 6):       # cols 2048..3072
                if FP8:
                    wts = load_wpair(wqkv, slab * 512, (slab + 1) * 512)
                else:
                    wts = [wpool.tile([P, 512], bf16, tag="wslab", name="wslab")
                           for _ in range(KT)]
                    for k in range(KT):
                        nc.sync.dma_start(
                            out=wts[k],
                            in_=wqkv[l, k * P:(k + 1) * P,
                                     slab * 512:(slab + 1) * 512])
                nh = slab - 4
                for s in range(BPC):
                    off = SOFF[s]
                    for mb in range(SJ[s]):
                        ps = psp.tile([P, 8, DH], f32, tag="mm", bufs=3, name="psmm")
                        if not zb:
                            nc.tensor.matmul(
                                ps, ones_row,
                                rowp[:, nh * 512:(nh + 1) * 512],
                                start=True, stop=False)
                        if FP8:
                            for pr in range(4):
                                nc.tensor.matmul(
                                    ps, hp[pr][:, :, off + mb * P:off + (mb + 1) * P],
                                    wts[pr][:, :, :],
                                    start=(zb and pr == 0), stop=(pr == 3),
                                    perf_mode=DR)
                            nc.vector.tensor_scalar_mul(
                                vtt[s][:, mb, nh * 8:(nh + 1) * 8, 0:DH],
                                ps, DQ)
                        else:
                            for k in range(KT):
                                nc.tensor.matmul(
                                    ps, htiles[k][:, off + mb * P:off + (mb + 1) * P],
                                    wts[k], start=(zb and k == 0),
                                    stop=(k == KT - 1))
                            nc.vector.tensor_copy(
                                out=vtt[s][:, mb, nh * 8:(nh + 1) * 8, 0:DH],
                                in_=ps)

            # ---------------- attention (per sample) ----------------
            if FP8:
                atp = [apool.tile([P, 2, CT], f8, tag=f"at{i}", name=f"at{i}")
                       for i in range(4)]
                asl = lambda k, po, lo, hi: atp[k // 2][po:po + DH, k % 2, lo:hi]
            else:
                attnT = [apool.tile([P, CT], bf16, tag=f"at{k}", name=f"at{k}") for k in range(KT)]
                asl = lambda k, po, lo, hi: attnT[k][po:po + DH, lo:hi]
            for s in range(BPC):
                off, T, J = SOFF[s], STOK[s], SJ[s]
                for hh in range(H):
                    band = hh // 2
                    po = (hh % 2) * DH
                    q_ap = qt[band][po:po + DH, off:off + T]
                    k_ap = kt_[band][po:po + DH, :]
                    etiles = []
                    for j in range(J):
                        e = epool.tile([P, 256 if j < 2 else 512], bf16,
                                       tag=f"e{j}", bufs=2, name=f"e{j}")
                        ks = k_ap[:, off + j * P:off + (j + 1) * P]
                        psS = psp.tile([P, 512], f32, tag="mm", bufs=3,
                                       name="psS")
                        if j < 2:
                            # mask-token keys: only mask queries, no mask
                            nc.tensor.matmul(psS[:, :M], ks, q_ap[:, :M],
                                             start=True, stop=True)
                            nc.scalar.activation(e[:, :M], psS[:, :M], AF.Exp,
                                                 bias=zerot)
                        else:
                            # full query span; Ni cutoff via DVE add on the
                            # mask-query region, causal triangle on the diag.
                            # (mid region of j>2 tiles is garbage, never read)
                            lc = latc[:, SLC[s] + (j - 2):SLC[s] + (j - 2) + 1]
                            dq = j * P        # diagonal query start
                            nc.tensor.matmul(psS[:, :T], ks, q_ap,
                                             start=True, stop=True)
                            nc.vector.tensor_scalar_add(
                                psS[:, :M], psS[:, :M], lc)
                            nc.vector.tensor_add(
                                psS[:, dq:dq + P], psS[:, dq:dq + P], trit)
                            nc.scalar.activation(e[:, :T], psS[:, :T],
                                                 AF.Exp, bias=zerot)
                        etiles.append(e)
                    # AV with folded denominator (ones col in vtt)
                    rs = spool.tile([1, 512], f32, tag="stat", name="rs")
                    rsb = rpool.tile([DH, 512], f32, tag="rsb", name="rsb")
                    # region [0, M): all key tiles
                    psA = psp.tile([DH + 1, M], f32, tag="OTa", bufs=2, name="psA")
                    for j in range(J):
                        nc.tensor.matmul(psA, vtt[s][:, j, hh, :],
                                         etiles[j][:, :M],
                                         start=(j == 0), stop=(j == J - 1))
                    nc.vector.reciprocal(rs[:, :M], psA[DH:DH + 1, :])
                    nc.gpsimd.partition_broadcast(rsb[:, :M], rs[:, :M])
                    nc.vector.tensor_mul(asl(band, po, off, off + M),
                                         psA[0:DH, :], rsb[:, :M])
                    # latent query chunks c: key tiles 2..2+c
                    for cch in range(J - 2):
                        qs = M + cch * P
                        pc = psp.tile([DH + 1, P], f32, tag="OTc", bufs=1,
                                      name=f"psC{cch}")
                        for j in range(2, 3 + cch):
                            nc.tensor.matmul(pc, vtt[s][:, j, hh, :],
                                             etiles[j][:, qs:qs + P],
                                             start=(j == 2), stop=(j == 2 + cch))
                        nc.vector.reciprocal(rs[:, qs:qs + P],
                                             pc[DH:DH + 1, :])
                        nc.gpsimd.partition_broadcast(rsb[:, qs:qs + P],
                                                      rs[:, qs:qs + P])
                        nc.vector.tensor_mul(asl(band, po, off + qs, off + qs + P),
                                             pc[0:DH, :], rsb[:, qs:qs + P])

            # ---------------- proj (+residual), hoisted ----------------
            for slab in range(2):
                if FP8:
                    wts = load_wpair(wproj, slab * 512, (slab + 1) * 512)
                else:
                    wts = [wpool.tile([P, 512], bf16, tag="wslab", name="wslab")
                           for _ in range(KT)]
                    for k in range(KT):
                        nc.sync.dma_start(
                            out=wts[k],
                            in_=wproj[l, k * P:(k + 1) * P,
                                      slab * 512:(slab + 1) * 512])
                for ni_ in range(4):
                    nt = slab * 4 + ni_
                    for s in range(BPC):
                        off, T = SOFF[s], STOK[s]
                        ps = psp.tile([P, 512], f32, tag="mm", bufs=3, name="psmm")
                        if FP8:
                            for pr in range(4):
                                nc.tensor.matmul(
                                    ps[:, :T], wts[pr][:, :, ni_ * P:(ni_ + 1) * P],
                                    atp[pr][:, :, off:off + T],
                                    start=(pr == 0), stop=(pr == 3),
                                    perf_mode=DR)
                            t_ = tpool.tile([P, 512], f32, tag="deq", name="deq")
                            nc.vector.tensor_scalar(
                                out=t_[:, :T], in0=ps[:, :T], scalar1=DQ,
                                scalar2=pt[:, PC_BPROJ + nt:PC_BPROJ + nt + 1],
                                op0=OP.mult, op1=OP.add)
                            nc.vector.tensor_add(
                                xt[nt][:, off:off + T], xt[nt][:, off:off + T],
                                t_[:, :T])
                        else:
                            for k in range(KT):
                                nc.tensor.matmul(
                                    ps[:, :T], wts[k][:, ni_ * P:(ni_ + 1) * P],
                                    attnT[k][:, off:off + T],
                                    start=(k == 0), stop=(k == KT - 1))
                            nc.vector.scalar_tensor_tensor(
                                out=xt[nt][:, off:off + T], in0=ps[:, :T],
                                scalar=pt[:, PC_BPROJ + nt:PC_BPROJ + nt + 1],
                                in1=xt[nt][:, off:off + T],
                                op0=OP.add, op1=OP.add)

            # ---------------- LN2 -> h ----------------
            if FP8:
                hp = [hpool.tile([P, 2, CT], f8, tag=f"h{i}", name=f"h{i}")
                      for i in range(4)]
                hsl = lambda k, lo, hi: hp[k // 2][:, k % 2, lo:hi]
            else:
                htiles = [hpool.tile([P, CT], bf16, tag=f"h{k}", name=f"h{k}") for k in range(KT)]
                hsl = lambda k, lo, hi: htiles[k][:, lo:hi]
            for s in range(BPC):
                emit_ln(SOFF[s], STOK[s], g2, bl2, hsl, SOFF[s])

            # ---------------- FF, hoisted ----------------
            for c in range(2):
                if FP8:
                    gp2 = [gpool.tile([P, 2, CT], f8, tag=f"g{i}", name=f"g{i}")
                           for i in range(8)]
                    gsl = lambda fi, lo, hi: gp2[fi // 2][:, fi % 2, lo:hi]
                else:
                    gt = [gpool.tile([P, CT], bf16, tag=f"g{i}", name=f"g{i}") for i in range(16)]
                    gsl = lambda fi, lo, hi: gt[fi][:, lo:hi]
                for slab in range(4):
                    co = c * 2048 + slab * 512
                    if FP8:
                        wts = load_wpair(w1, co, co + 512)
                    else:
                        wts = [wpool.tile([P, 512], bf16, tag="wslab", name="wslab")
                               for _ in range(KT)]
                        for k in range(KT):
                            nc.sync.dma_start(
                                out=wts[k],
                                in_=w1[l, k * P:(k + 1) * P, co:co + 512])
                    for ni_ in range(4):
                        fi = slab * 4 + ni_           # within chunk c
                        bc = PC_B1 + c * 16 + fi
                        for s in range(BPC):
                            off, T = SOFF[s], STOK[s]
                            ps = psp.tile([P, 512], f32, tag="mm", bufs=3, name="psmm")
                            chainA(ps[:, :T], wts, ni_ * P, off, T)
                            nc.scalar.activation(gsl(fi, off, off + T),
                                                 ps[:, :T], AF.Gelu,
                                                 bias=pt[:, bc:bc + 1],
                                                 scale=(DQ if FP8 else 1.0))
                for nslab in range(2):
                    if FP8:
                        w2t = [w2pool.tile([P, 2, 512], f8, tag="w2slab",
                                           name="w2slab") for _ in range(8)]
                        for pr in range(8):
                            for hf in range(2):
                                ro = c * 2048 + (2 * pr + hf) * P
                                nc.sync.dma_start(
                                    out=w2t[pr][:, hf, :],
                                    in_=w2[l, ro:ro + P,
                                           nslab * 512:(nslab + 1) * 512])
                    else:
                        w2t = [w2pool.tile([P, 512], bf16, tag="w2slab", name="w2slab")
                               for _ in range(16)]
                        for k2 in range(16):
                            ro = c * 2048 + k2 * P
                            nc.sync.dma_start(
                                out=w2t[k2],
                                in_=w2[l, ro:ro + P,
                                       nslab * 512:(nslab + 1) * 512])
                    for ni_ in range(4):
                        nt = nslab * 4 + ni_
                        sc_ap = (pt[:, PC_B2 + nt:PC_B2 + nt + 1]
                                 if c == 1 else 0.0)
                        for s in range(BPC):
                            off, T = SOFF[s], STOK[s]
                            ps = psp.tile([P, 512], f32, tag="mm", bufs=3, name="psmm")
                            if FP8:
                                for pr in range(8):
                                    nc.tensor.matmul(
                                        ps[:, :T],
                                        w2t[pr][:, :, ni_ * P:(ni_ + 1) * P],
                                        gp2[pr][:, :, off:off + T],
                                        start=(pr == 0), stop=(pr == 7),
                                        perf_mode=DR)
                                t_ = tpool.tile([P, 512], f32, tag="deq", name="deq")
                                nc.vector.tensor_scalar(
                                    out=t_[:, :T], in0=ps[:, :T], scalar1=DQ,
                                    scalar2=sc_ap, op0=OP.mult, op1=OP.add)
                                nc.vector.tensor_add(
                                    xt[nt][:, off:off + T],
                                    xt[nt][:, off:off + T], t_[:, :T])
                            else:
                                for k2 in range(16):
                                    nc.tensor.matmul(
                                        ps[:, :T], w2t[k2][:, ni_ * P:(ni_ + 1) * P],
                                        gt[k2][:, off:off + T],
                                        start=(k2 == 0), stop=(k2 == 15))
                                nc.vector.scalar_tensor_tensor(
                                    out=xt[nt][:, off:off + T], in0=ps[:, :T],
                                    scalar=sc_ap,
                                    in1=xt[nt][:, off:off + T],
                                    op0=OP.add, op1=OP.add)

        # ---------------- epilogue: ln_post + out proj ----------------
        gp = [postt[:, k:k + 1] for k in range(KT)]
        bp = [postt[:, 8 + k:8 + k + 1] for k in range(KT)]
        for s in range(BPC):
            hE = [ypool.tile([P, M], bf16, tag=f"hE{k}", name=f"hE{k}")
                  for k in range(KT)]
            hsl = lambda k, lo, hi: hE[k][:, lo - SOFF[s]:hi - SOFF[s]]
            emit_ln(SOFF[s], M, gp, bp, hsl, SOFF[s])
            pso = psp.tile([VAE, M], f32, tag="row", bufs=2, name="psout")
            for k in range(KT):
                nc.tensor.matmul(pso, woutt[:, k * VAE:(k + 1) * VAE],
                                 hE[k][:, :M],
                                 start=(k == 0), stop=(k == KT - 1))
            y = ypool.tile([VAE, M], f32, tag="y", name="y")
            nc.vector.tensor_scalar_add(y, pso, boutt)
            nc.sync.dma_start(out=out_d[s].rearrange("r c -> c r"), in_=y)

    nc.finalize()
    return nc


def kernel(**inputs):
    global _PROG, _PROG_KEY
    from concourse.bass_utils import run_bass_kernel_spmd
    in_maps, order, TA, TB = _host_prep(inputs)
    zb = not np.any(inputs["qkv_b"][:, 2 * D:])
    if _PROG is None or _PROG_KEY != (TA, TB, zb):
        _PROG = _build_bass(TA, TB, zb)
        _PROG_KEY = (TA, TB, zb)
    res = run_bass_kernel_spmd(_PROG, in_maps, list(range(NCORES)))
    out = np.zeros((B, M, VAE), np.float32)
    for c in range(NCORES):
        o = res.results[c]["out"]
        out[int(order[c])] = o[0]
        out[int(order[NCORES + c])] = o[1]
    return out.astype(np.float32)


# revision 14
# speedup vs baseline: 1.0362x; 1.0362x over previous
"""Trainium2 Bass kernel for nn_Decoder_1D_Matryoshka (12-layer masked decoder).

v2: exploits the Matryoshka sparsity. Rows >= 256+Ni are dead (never attended
by any row that reaches the output), so each sample only needs T_i = 256+Ni
tokens. Samples are split into two shape classes (TA = 8 largest, TB = rest),
one of each per core; per-sample masks/zero-padding carry correctness so the
SPMD program is identical across cores. Attention exploits the block mask
structure: mask-token keys are only attended by mask-token queries, latent
keys only by causally-later latent queries + all mask queries. The Ni cutoff
is a per-key-partition exp bias; only the 128x128 causal diagonal needs a
tensor mask add (static triangle). Softmax denominators come free from the
AV matmul via an interleaved ones column in V. Weight DMAs are hoisted so
each layer's weights stream once (not once per sample).

Residual stream kept transposed (features on partitions, tokens free) as in
v1; attention uses transposed scores S^T = K Q^T.
"""

import os
import numpy as np
import ml_dtypes

B, NLAT, DIN = 16, 256, 32
D, H, NL, VAE = 1024, 16, 12, 16
M = 256
DH = 64
NCORES = 8
BPC = 2
P = 128
KT = D // P       # 8 k-bands over D
FF = 4 * D
NL_OVR = int(os.environ.get("BASSK_NL", NL))

BF16 = ml_dtypes.bfloat16
FP8NP = ml_dtypes.float8_e4m3
FP8 = os.environ.get("BASSK_FP8", "0") == "1"
WS = 64.0         # fp8 weight scale

# param-tile column layout (one [128, 104] f32 DMA per layer)
PC_BQKV = 0       # 24 cols: qkv bias, n-tile t at col t (q cols pre-scaled)
PC_BPROJ = 24     # 8
PC_B1 = 32        # 32
PC_B2 = 64        # 8
PC_G1 = 72        # 8
PC_BL1 = 80       # 8
PC_G2 = 88        # 8
PC_BL2 = 96       # 8
PCOLS = 104


def _layernorm_np(x, g, b, eps=1e-5):
    mu = x.mean(-1, keepdims=True)
    var = ((x - mu) ** 2).mean(-1, keepdims=True)
    return (x - mu) / np.sqrt(var + eps) * g + b


def _classes(ni):
    T = 256 + np.asarray(ni).astype(np.int64)
    Tp = ((T + 127) // 128) * 128
    order = np.argsort(-Tp, kind="stable")
    TA = int(Tp[order[0]])
    TB = int(Tp[order[NCORES]])
    return order, TA, TB


def _host_prep(inputs):
    """Returns (per_core_in_maps, order, TA, TB)."""
    f32 = np.float32
    ni = np.asarray(inputs["num_activated"]).astype(np.int64)
    order, TA, TB = _classes(ni)
    CT = TA + TB

    lat = inputs["latents"].astype(f32)
    x_lat = lat.reshape(B * NLAT, DIN) @ inputs["input_w"].astype(f32)
    x_lat = x_lat.reshape(B, NLAT, D) + inputs["input_b"][None, None, :]
    x_lat = x_lat + inputs["latents_pos_embed"][None, :, :]
    mt = inputs["mask_tokens"].reshape(1, 1, D) + inputs["pos_embed_full"]
    mt = np.broadcast_to(mt, (B, M, D))
    x = np.concatenate([mt, x_lat], axis=1)                       # (B, L, D)
    x = _layernorm_np(x, inputs["ln_pre_g"], inputs["ln_pre_b"]).astype(f32)

    # static causal triangle for the diagonal 128x128 latent blocks
    # tri[key, query] = -30 where key > query
    rr = np.arange(P)
    tri = np.where(rr[:, None] <= rr[None, :], f32(0.0), f32(-30.0))

    # ---- static (identical on every core) weights, repacked ----
    scale = np.float32(DH ** -0.5)
    wqkv = inputs["qkv_w"].astype(f32).copy()                      # (NL,D,3D)
    wqkv[:, :, :D] *= scale
    bqkv = inputs["qkv_b"].astype(f32).copy()                      # (NL,3D)
    bqkv[:, :D] *= scale

    params = np.zeros((NL, P, PCOLS), f32)
    params[:, :, PC_BQKV:PC_BQKV + 24] = bqkv.reshape(NL, 24, P).transpose(0, 2, 1)
    params[:, :, PC_BPROJ:PC_BPROJ + 8] = inputs["proj_b"].reshape(NL, 8, P).transpose(0, 2, 1)
    params[:, :, PC_B1:PC_B1 + 32] = inputs["fc1_b"].reshape(NL, 32, P).transpose(0, 2, 1)
    params[:, :, PC_B2:PC_B2 + 8] = inputs["fc2_b"].reshape(NL, 8, P).transpose(0, 2, 1)
    params[:, :, PC_G1:PC_G1 + 8] = inputs["ln1_g"].reshape(NL, 8, P).transpose(0, 2, 1)
    params[:, :, PC_BL1:PC_BL1 + 8] = inputs["ln1_b"].reshape(NL, 8, P).transpose(0, 2, 1)
    params[:, :, PC_G2:PC_G2 + 8] = inputs["ln2_g"].reshape(NL, 8, P).transpose(0, 2, 1)
    params[:, :, PC_BL2:PC_BL2 + 8] = inputs["ln2_b"].reshape(NL, 8, P).transpose(0, 2, 1)

    # v-bias rows, seeded into psum via K=1 ones-matmul: (NL, 1, D)
    rowparams = bqkv[:, 2 * D:3 * D].reshape(NL, 1, D).astype(f32)

    postparams = np.zeros((P, 16), f32)
    postparams[:, 0:8] = inputs["ln_post_g"].reshape(8, P).T
    postparams[:, 8:16] = inputs["ln_post_b"].reshape(8, P).T

    if FP8:
        def q8(w):
            return np.ascontiguousarray(
                np.clip(w.astype(f32) * WS, -240, 240).astype(FP8NP))
        wmain = {
            "wqkv": q8(wqkv),
            "wproj": q8(inputs["proj_w"]),
            "w1": q8(inputs["fc1_w"]),
            "w2": q8(inputs["fc2_w"]),
        }
        rowparams = rowparams * np.float32(WS)
    else:
        wmain = {
            "wqkv": np.ascontiguousarray(wqkv.astype(BF16)),
            "wproj": np.ascontiguousarray(inputs["proj_w"].astype(BF16)),
            "w1": np.ascontiguousarray(inputs["fc1_w"].astype(BF16)),
            "w2": np.ascontiguousarray(inputs["fc2_w"].astype(BF16)),
        }
    static = {
        **wmain,
        "params": params,
        "rowparams": np.ascontiguousarray(rowparams.astype(BF16)),
        "postparams": postparams,
        "tri": np.ascontiguousarray(tri.astype(BF16)),
        "wout": np.ascontiguousarray(
            inputs["out_w"].astype(BF16).reshape(KT, P, VAE)),
        "bout": inputs["out_b"].astype(f32).reshape(VAE, 1),
    }

    in_maps = []
    for c in range(NCORES):
        sA, sB = int(order[c]), int(order[NCORES + c])
        x0T = np.zeros((D, CT), f32)
        x0T[:, :256 + ni[sA]] = x[sA, :256 + ni[sA]].T
        x0T[:, TA:TA + 256 + ni[sB]] = x[sB, :256 + ni[sB]].T
        # per-latent-tile cutoff bias: cols 0..1 sample A tiles, 2..3 sample B
        latcut = np.full((P, 4), f32(-30.0))
        for j in range((TA - 256) // P):
            latcut[:, j] = np.where(j * P + rr < ni[sA], f32(0.0), f32(-30.0))
        for j in range((TB - 256) // P):
            latcut[:, 2 + j] = np.where(j * P + rr < ni[sB], f32(0.0), f32(-30.0))
        m = {"x0T": x0T, "latcut": latcut}
        m.update(static)
        in_maps.append(m)
    return in_maps, order, TA, TB


_PROG = None
_PROG_KEY = None


def _build_bass(TA, TB, zb):
    import concourse.bass as bass
    import concourse.bacc as bacc
    import concourse.tile as tile
    import concourse.mybir as mybir
    from contextlib import ExitStack

    f32 = mybir.dt.float32
    bf16 = mybir.dt.bfloat16
    f8 = mybir.dt.float8e4
    wdt = f8 if FP8 else bf16
    DR = mybir.MatmulPerfMode.DoubleRow if FP8 else None
    DQ = 1.0 / WS
    AF = mybir.ActivationFunctionType
    OP = mybir.AluOpType
    NLn = NL_OVR
    CT = TA + TB
    JA = TA // P          # key tiles sample A (4 when TA=512)
    JB = TB // P
    SOFF = (0, TA)
    STOK = (TA, TB)
    SJ = (JA, JB)
    SLC = (0, 2)          # latcut col base per sample

    nc = bacc.Bacc(None, target_bir_lowering=False, debug=False)
    dp = nc.declare_dram_parameter
    x0T = dp("x0T", [D, CT], f32, isOutput=False)
    latcut_d = dp("latcut", [P, 4], f32, isOutput=False)
    tri_d = dp("tri", [P, P], bf16, isOutput=False)
    wqkv = dp("wqkv", [NL, D, 3 * D], wdt, isOutput=False)
    wproj = dp("wproj", [NL, D, D], wdt, isOutput=False)
    w1 = dp("w1", [NL, D, FF], wdt, isOutput=False)
    w2 = dp("w2", [NL, FF, D], wdt, isOutput=False)
    params = dp("params", [NL, P, PCOLS], f32, isOutput=False)
    rowparams = dp("rowparams", [NL, 1, D], bf16, isOutput=False)
    postparams = dp("postparams", [P, 16], f32, isOutput=False)
    wout_d = dp("wout", [KT, P, VAE], bf16, isOutput=False)
    bout_d = dp("bout", [VAE, 1], f32, isOutput=False)
    out_d = dp("out", [BPC, M, VAE], f32, isOutput=True)

    with ExitStack() as ctx:
        tc = ctx.enter_context(tile.TileContext(nc))
        # ---- pools ----
        persist = ctx.enter_context(tc.tile_pool(name="persist", bufs=1))
        wpool = ctx.enter_context(tc.tile_pool(name="wslab", bufs=16))
        w2pool = ctx.enter_context(tc.tile_pool(name="w2slab", bufs=18))
        hpool = ctx.enter_context(tc.tile_pool(name="h", bufs=1))
        qkpool = ctx.enter_context(tc.tile_pool(name="qk", bufs=1))
        apool = ctx.enter_context(tc.tile_pool(name="attnT", bufs=1))
        gpool = ctx.enter_context(tc.tile_pool(name="g", bufs=1))
        epool = ctx.enter_context(tc.tile_pool(name="expS", bufs=2))
        tpool = ctx.enter_context(tc.tile_pool(name="scratch", bufs=2))
        spool = ctx.enter_context(tc.tile_pool(name="stats", bufs=5))
        rpool = ctx.enter_context(tc.tile_pool(name="rsb", bufs=2))
        bpool = ctx.enter_context(tc.tile_pool(name="lnb", bufs=1))
        ppool = ctx.enter_context(tc.tile_pool(name="lparams", bufs=2))
        ypool = ctx.enter_context(tc.tile_pool(name="yout", bufs=1))
        psp = ctx.enter_context(tc.tile_pool(name="ps", bufs=1, space="PSUM"))

        # ---- persistent tiles ----
        xt = [persist.tile([P, CT], f32, tag=f"x{k}", name=f"x{k}") for k in range(KT)]
        ones_f = persist.tile([P, 1], f32, tag="ones_f", name="ones_f")
        ones_b = persist.tile([P, 1], bf16, tag="ones_b", name="ones_b")
        ones_row = persist.tile([1, P], bf16, tag="ones_row", name="ones_row")
        trit = persist.tile([P, P], bf16, tag="tri", name="tri")
        latc = persist.tile([P, 4], f32, tag="latc", name="latc")
        # V with interleaved ones column per head: [P, J, 16 heads, 65]
        vtt = [persist.tile([P, SJ[s], H, DH + 1], bf16, tag=f"vt{s}", name=f"vt{s}")
               for s in range(BPC)]
        woutt = persist.tile([P, KT * VAE], bf16, tag="wo", name="wo")
        boutt = persist.tile([VAE, 1], f32, tag="bout", name="bout")
        postt = persist.tile([P, 16], f32, tag="post", name="post")
        epst = persist.tile([1, 1], f32, tag="eps", name="eps")
        nc.vector.memset(epst, 1e-5)
        zerot = persist.tile([P, 1], f32, tag="zerot", name="zerot")
        nc.vector.memset(zerot, 0.0)

        for k in range(KT):
            nc.sync.dma_start(out=xt[k], in_=x0T[k * P:(k + 1) * P, :])
            nc.sync.dma_start(out=woutt[:, k * VAE:(k + 1) * VAE], in_=wout_d[k])
        nc.vector.memset(ones_f, 1.0)
        nc.vector.memset(ones_b, 1.0)
        nc.vector.memset(ones_row, 1.0)
        for s in range(BPC):
            nc.vector.memset(vtt[s], 1.0)
        nc.sync.dma_start(out=trit, in_=tri_d[:, :])
        nc.sync.dma_start(out=latc, in_=latcut_d[:, :])
        nc.sync.dma_start(out=boutt, in_=bout_d[:, :])
        nc.sync.dma_start(out=postt, in_=postparams[:, :])

        def emit_ln(src_off, width, g_ap, b_ap, hsl, h_off, xq=None):
            """LN over features (partitions) of x[:, src_off:src_off+width] ->
            hsl(k, h_off, h_off+width). xq: optional precomputed full-width
            (xb, xsq) bf16 tiles."""
            psx = psp.tile([1, 512], f32, tag="row", bufs=2, name="psx")
            psxx = psp.tile([1, 512], f32, tag="row", bufs=2, name="psxx")
            for k in range(KT):
                xs = xt[k][:, src_off:src_off + width]
                if xq is None:
                    xsq = tpool.tile([P, 512], bf16, tag="xsq", name="xsq")
                    xb = tpool.tile([P, 512], bf16, tag="xb", name="xb")
                    nc.scalar.activation(xsq[:, :width], xs, AF.Square)
                    nc.vector.tensor_copy(out=xb[:, :width], in_=xs)
                    xb_ap = xb[:, :width]
                    xsq_ap = xsq[:, :width]
                else:
                    xb_ap = xq[0][k][:, src_off:src_off + width]
                    xsq_ap = xq[1][k][:, src_off:src_off + width]
                nc.tensor.matmul(psx[:, :width], ones_b, xb_ap,
                                 start=(k == 0), stop=(k == KT - 1))
                nc.tensor.matmul(psxx[:, :width], ones_b, xsq_ap,
                                 start=(k == 0), stop=(k == KT - 1))
            mu = spool.tile([1, 512], f32, tag="stat", name="mu")
            e2 = spool.tile([1, 512], f32, tag="stat", name="e2")
            rstd = spool.tile([1, 512], f32, tag="stat", name="rstd")
            nc.vector.tensor_scalar_mul(mu[:, :width], psx[:, :width], 1.0 / D)
            nc.vector.tensor_scalar_mul(e2[:, :width], psxx[:, :width], 1.0 / D)
            musq = spool.tile([1, 512], f32, tag="stat", name="musq")
            nc.vector.tensor_mul(musq[:, :width], mu[:, :width], mu[:, :width])
            nc.vector.tensor_sub(e2[:, :width], e2[:, :width], musq[:, :width])
            nc.scalar.activation(e2[:, :width], e2[:, :width], AF.Sqrt,
                                 bias=epst)
            nc.vector.reciprocal(rstd[:, :width], e2[:, :width])
            pmu = bpool.tile([P, 512], f32, tag="pmu", name="pmu")
            prs = bpool.tile([P, 512], f32, tag="prs", name="prs")
            nc.gpsimd.partition_broadcast(pmu[:, :width], mu[:, :width])
            nc.gpsimd.partition_broadcast(prs[:, :width], rstd[:, :width])
            for k in range(KT):
                xs = xt[k][:, src_off:src_off + width]
                t = tpool.tile([P, 512], f32, tag="lnt", name="lnt")
                nc.vector.tensor_sub(t[:, :width], xs, pmu[:, :width])
                nc.vector.tensor_mul(t[:, :width], t[:, :width], prs[:, :width])
                nc.vector.tensor_scalar(
                    out=hsl(k, h_off, h_off + width), in0=t[:, :width],
                    scalar1=g_ap[k], scalar2=b_ap[k],
                    op0=OP.mult, op1=OP.add)

        for l in range(NLn):
            pt = ppool.tile([P, PCOLS], f32, tag="pt", name="pt")
            rowp = ppool.tile([1, D], bf16, tag="rowp", name="rowp")
            nc.sync.dma_start(out=pt, in_=params[l])
            nc.sync.dma_start(out=rowp, in_=rowparams[l])
            g1 = [pt[:, PC_G1 + k:PC_G1 + k + 1] for k in range(KT)]
            bl1 = [pt[:, PC_BL1 + k:PC_BL1 + k + 1] for k in range(KT)]
            g2 = [pt[:, PC_G2 + k:PC_G2 + k + 1] for k in range(KT)]
            bl2 = [pt[:, PC_BL2 + k:PC_BL2 + k + 1] for k in range(KT)]

            # ---------------- LN1 -> h (both samples) ----------------
            if FP8:
                hp = [hpool.tile([P, 2, CT], f8, tag=f"h{i}", name=f"h{i}")
                      for i in range(4)]
                hsl = lambda k, lo, hi: hp[k // 2][:, k % 2, lo:hi]
            else:
                htiles = [hpool.tile([P, CT], bf16, tag=f"h{k}", name=f"h{k}") for k in range(KT)]
                hsl = lambda k, lo, hi: htiles[k][:, lo:hi]
            for s in range(BPC):
                emit_ln(SOFF[s], STOK[s], g1, bl1, hsl, SOFF[s])

            def load_wpair(src_d, cl, ch):
                wts = [wpool.tile([P, 2, ch - cl], f8, tag="wslab",
                                  name="wslab") for _ in range(4)]
                for pr in range(4):
                    for hf in range(2):
                        nc.sync.dma_start(
                            out=wts[pr][:, hf, :],
                            in_=src_d[l, (2 * pr + hf) * P:(2 * pr + hf + 1) * P,
                                      cl:ch])
                return wts

            def chainA(ps_ap, wts, col, off, T, seed=None):
                if seed is not None:
                    nc.tensor.matmul(ps_ap, ones_row, seed,
                                     start=True, stop=False)
                if FP8:
                    for pr in range(4):
                        nc.tensor.matmul(
                            ps_ap, wts[pr][:, :, col:col + P],
                            hp[pr][:, :, off:off + T],
                            start=(pr == 0 and seed is None), stop=(pr == 3),
                            perf_mode=DR)
                else:
                    for k in range(KT):
                        nc.tensor.matmul(
                            ps_ap, wts[k][:, col:col + P],
                            htiles[k][:, off:off + T],
                            start=(k == 0 and seed is None), stop=(k == KT - 1))

            # ---------------- QKV/attn/proj with cross-sample overlap ----
            qt = [qkpool.tile([P, CT], bf16, tag=f"q{n}", name=f"q{n}") for n in range(KT)]
            kt_ = [qkpool.tile([P, CT], bf16, tag=f"k{n}", name=f"k{n}") for n in range(KT)]
            if FP8:
                atp = [apool.tile([P, 2, CT], f8, tag=f"at{i}", name=f"at{i}")
                       for i in range(4)]
                asl = lambda k, po, lo, hi: atp[k // 2][po:po + DH, k % 2, lo:hi]
            else:
                attnT = [apool.tile([P, CT], bf16, tag=f"at{k}", name=f"at{k}") for k in range(KT)]
                asl = lambda k, po, lo, hi: attnT[k][po:po + DH, lo:hi]

            def qkv_units(s):
                off, T = SOFF[s], STOK[s]
                units = []
                wstore = {}
                for slab in range(4):      # cols 0..2048 of wqkv: q then k
                    def load(slab=slab):
                        if FP8:
                            wstore[slab] = load_wpair(
                                wqkv, slab * 512, (slab + 1) * 512)
                        else:
                            wts = [wpool.tile([P, 512], bf16, tag="wslab",
                                              name="wslab") for _ in range(KT)]
                            for k in range(KT):
                                nc.sync.dma_start(
                                    out=wts[k],
                                    in_=wqkv[l, k * P:(k + 1) * P,
                                             slab * 512:(slab + 1) * 512])
                            wstore[slab] = wts
                    units.append(load)
                    for ni_ in range(4):
                        def chain(slab=slab, ni_=ni_):
                            nt = slab * 4 + ni_
                            dst = qt[nt] if nt < KT else kt_[nt - KT]
                            ps = psp.tile([P, 512], f32, tag="mm", bufs=3,
                                          name="psmm")
                            chainA(ps[:, :T], wstore[slab], ni_ * P, off, T)
                            if FP8:
                                nc.vector.tensor_scalar(
                                    out=dst[:, off:off + T], in0=ps[:, :T],
                                    scalar1=DQ,
                                    scalar2=pt[:, PC_BQKV + nt:PC_BQKV + nt + 1],
                                    op0=OP.mult, op1=OP.add)
                            else:
                                nc.vector.tensor_scalar_add(
                                    dst[:, off:off + T], ps[:, :T],
                                    pt[:, PC_BQKV + nt:PC_BQKV + nt + 1])
                        units.append(chain)
                for slab in range(4, 6):   # v cols 2048..3072
                    def load(slab=slab):
                        if FP8:
                            wstore[slab] = load_wpair(
                                wqkv, slab * 512, (slab + 1) * 512)
                        else:
                            wts = [wpool.tile([P, 512], bf16, tag="wslab",
                                              name="wslab") for _ in range(KT)]
                            for k in range(KT):
                                nc.sync.dma_start(
                                    out=wts[k],
                                    in_=wqkv[l, k * P:(k + 1) * P,
                                             slab * 512:(slab + 1) * 512])
                            wstore[slab] = wts
                    units.append(load)
                    for mb in range(SJ[s]):
                        def vchain(slab=slab, mb=mb):
                            nh = slab - 4
                            wts = wstore[slab]
                            ps = psp.tile([P, 8, DH], f32, tag="mm", bufs=3,
                                          name="psmm")
                            if not zb:
                                nc.tensor.matmul(
                                    ps, ones_row,
                                    rowp[:, nh * 512:(nh + 1) * 512],
                                    start=True, stop=False)
                            if FP8:
                                for pr in range(4):
                                    nc.tensor.matmul(
                                        ps, hp[pr][:, :, off + mb * P:off + (mb + 1) * P],
                                        wts[pr][:, :, :],
                                        start=(zb and pr == 0), stop=(pr == 3),
                                        perf_mode=DR)
                                nc.vector.tensor_scalar_mul(
                                    vtt[s][:, mb, nh * 8:(nh + 1) * 8, 0:DH],
                                    ps, DQ)
                            else:
                                for k in range(KT):
                                    nc.tensor.matmul(
                                        ps, htiles[k][:, off + mb * P:off + (mb + 1) * P],
                                        wts[k], start=(zb and k == 0),
                                        stop=(k == KT - 1))
                                nc.vector.tensor_copy(
                                    out=vtt[s][:, mb, nh * 8:(nh + 1) * 8, 0:DH],
                                    in_=ps)
                        units.append(vchain)
                return units

            pwstore = {}

            def proj_units():
                units = []
                for slab in range(2):
                    def load(slab=slab):
                        if FP8:
                            pwstore[slab] = load_wpair(
                                wproj, slab * 512, (slab + 1) * 512)
                        else:
                            wts = [wpool.tile([P, 512], bf16, tag="wslab",
                                              name="wslab") for _ in range(KT)]
                            for k in range(KT):
                                nc.sync.dma_start(
                                    out=wts[k],
                                    in_=wproj[l, k * P:(k + 1) * P,
                                              slab * 512:(slab + 1) * 512])
                            pwstore[slab] = wts
                    units.append(load)
                return units

            def proj_chain(s, slab, ni_):
                off, T = SOFF[s], STOK[s]
                nt = slab * 4 + ni_
                wts = pwstore[slab]
                ps = psp.tile([P, 512], f32, tag="mm", bufs=3, name="psmm")
                if FP8:
                    for pr in range(4):
                        nc.tensor.matmul(
                            ps[:, :T], wts[pr][:, :, ni_ * P:(ni_ + 1) * P],
                            atp[pr][:, :, off:off + T],
                            start=(pr == 0), stop=(pr == 3),
                            perf_mode=DR)
                    t_ = tpool.tile([P, 512], f32, tag="deq", name="deq")
                    nc.vector.tensor_scalar(
                        out=t_[:, :T], in0=ps[:, :T], scalar1=DQ,
                        scalar2=pt[:, PC_BPROJ + nt:PC_BPROJ + nt + 1],
                        op0=OP.mult, op1=OP.add)
                    nc.vector.tensor_add(
                        xt[nt][:, off:off + T], xt[nt][:, off:off + T],
                        t_[:, :T])
                else:
                    for k in range(KT):
                        nc.tensor.matmul(
                            ps[:, :T], wts[k][:, ni_ * P:(ni_ + 1) * P],
                            attnT[k][:, off:off + T],
                            start=(k == 0), stop=(k == KT - 1))
                    nc.vector.scalar_tensor_tensor(
                        out=xt[nt][:, off:off + T], in0=ps[:, :T],
                        scalar=pt[:, PC_BPROJ + nt:PC_BPROJ + nt + 1],
                        in1=xt[nt][:, off:off + T],
                        op0=OP.add, op1=OP.add)

            def attn_head(s, hh):
                off, T, J = SOFF[s], STOK[s], SJ[s]
                band = hh // 2
                po = (hh % 2) * DH
                q_ap = qt[band][po:po + DH, off:off + T]
                k_ap = kt_[band][po:po + DH, :]
                etiles = []
                for j in range(J):
                    e = epool.tile([P, 256 if j < 2 else 512], bf16,
                                   tag=f"e{j}", bufs=2, name=f"e{j}")
                    ks = k_ap[:, off + j * P:off + (j + 1) * P]
                    psS = psp.tile([P, 512], f32, tag="mm", bufs=3,
                                   name="psS")
                    if j < 2:
                        nc.tensor.matmul(psS[:, :M], ks, q_ap[:, :M],
                                         start=True, stop=True)
                        nc.scalar.activation(e[:, :M], psS[:, :M], AF.Exp,
                                             bias=zerot)
                    else:
                        lc = latc[:, SLC[s] + (j - 2):SLC[s] + (j - 2) + 1]
                        dq = j * P
                        nc.tensor.matmul(psS[:, :T], ks, q_ap,
                                         start=True, stop=True)
                        nc.vector.tensor_scalar_add(
                            psS[:, :M], psS[:, :M], lc)
                        nc.vector.tensor_add(
                            psS[:, dq:dq + P], psS[:, dq:dq + P], trit)
                        nc.scalar.activation(e[:, :T], psS[:, :T],
                                             AF.Exp, bias=zerot)
                    etiles.append(e)
                rs = spool.tile([1, 512], f32, tag="stat", name="rs")
                rsb = rpool.tile([DH, 512], f32, tag="rsb", name="rsb")
                psA = psp.tile([DH + 1, M], f32, tag="OTa", bufs=2, name="psA")
                for j in range(J):
                    nc.tensor.matmul(psA, vtt[s][:, j, hh, :],
                                     etiles[j][:, :M],
                                     start=(j == 0), stop=(j == J - 1))
                nc.vector.reciprocal(rs[:, :M], psA[DH:DH + 1, :])
                nc.gpsimd.partition_broadcast(rsb[:, :M], rs[:, :M])
                nc.vector.tensor_mul(asl(band, po, off, off + M),
                                     psA[0:DH, :], rsb[:, :M])
                for cch in range(J - 2):
                    qs = M + cch * P
                    pc = psp.tile([DH + 1, P], f32, tag="OTc", bufs=1,
                                  name=f"psC{cch}")
                    for j in range(2, 3 + cch):
                        nc.tensor.matmul(pc, vtt[s][:, j, hh, :],
                                         etiles[j][:, qs:qs + P],
                                         start=(j == 2), stop=(j == 2 + cch))
                    nc.vector.reciprocal(rs[:, qs:qs + P],
                                         pc[DH:DH + 1, :])
                    nc.gpsimd.partition_broadcast(rsb[:, qs:qs + P],
                                                  rs[:, qs:qs + P])
                    nc.vector.tensor_mul(asl(band, po, off + qs, off + qs + P),
                                         pc[0:DH, :], rsb[:, qs:qs + P])

            # emit: QKV(A); attn(A) fed by QKV(B) units; attn(B) fed by
            # proj loads + proj(A) chains; then proj(B).
            for u in qkv_units(0):
                u()
            fill = qkv_units(1)
            fi_ = 0
            for hh in range(H):
                take = (len(fill) * (hh + 1)) // H
                while fi_ < take:
                    fill[fi_](); fi_ += 1
                attn_head(0, hh)
            while fi_ < len(fill):
                fill[fi_](); fi_ += 1
            pfill = proj_units() + [
                (lambda slab=slab, ni_=ni_: proj_chain(0, slab, ni_))
                for slab in range(2) for ni_ in range(4)]
            fi_ = 0
            for hh in range(H):
                take = (len(pfill) * (hh + 1)) // H
                while fi_ < take:
                    pfill[fi_](); fi_ += 1
                attn_head(1, hh)
            while fi_ < len(pfill):
                pfill[fi_](); fi_ += 1
            for slab in range(2):
                for ni_ in range(4):
                    proj_chain(1, slab, ni_)

            # ---------------- LN2 -> h ----------------
            if FP8:
                hp = [hpool.tile([P, 2, CT], f8, tag=f"h{i}", name=f"h{i}")
                      for i in range(4)]
                hsl = lambda k, lo, hi: hp[k // 2][:, k % 2, lo:hi]
            else:
                htiles = [hpool.tile([P, CT], bf16, tag=f"h{k}", name=f"h{k}") for k in range(KT)]
                hsl = lambda k, lo, hi: htiles[k][:, lo:hi]
            for s in range(BPC):
                emit_ln(SOFF[s], STOK[s], g2, bl2, hsl, SOFF[s])

            # ---------------- FF, hoisted ----------------
            for c in range(2):
                if FP8:
                    gp2 = [gpool.tile([P, 2, CT], f8, tag=f"g{i}", name=f"g{i}")
                           for i in range(8)]
                    gsl = lambda fi, lo, hi: gp2[fi // 2][:, fi % 2, lo:hi]
                else:
                    gt = [gpool.tile([P, CT], bf16, tag=f"g{i}", name=f"g{i}") for i in range(16)]
                    gsl = lambda fi, lo, hi: gt[fi][:, lo:hi]
                for slab in range(4):
                    co = c * 2048 + slab * 512
                    if FP8:
                        wts = load_wpair(w1, co, co + 512)
                    else:
                        wts = [wpool.tile([P, 512], bf16, tag="wslab", name="wslab")
                               for _ in range(KT)]
                        for k in range(KT):
                            nc.sync.dma_start(
                                out=wts[k],
                                in_=w1[l, k * P:(k + 1) * P, co:co + 512])
                    for ni_ in range(4):
                        fi = slab * 4 + ni_           # within chunk c
                        bc = PC_B1 + c * 16 + fi
                        for s in range(BPC):
                            off, T = SOFF[s], STOK[s]
                            ps = psp.tile([P, 512], f32, tag="mm", bufs=3, name="psmm")
                            chainA(ps[:, :T], wts, ni_ * P, off, T)
                            nc.scalar.activation(gsl(fi, off, off + T),
                                                 ps[:, :T], AF.Gelu,
                                                 bias=pt[:, bc:bc + 1],
                                                 scale=(DQ if FP8 else 1.0))
                for nslab in range(2):
                    if FP8:
                        w2t = [w2pool.tile([P, 2, 512], f8, tag="w2slab",
                                           name="w2slab") for _ in range(8)]
                        for pr in range(8):
                            for hf in range(2):
                                ro = c * 2048 + (2 * pr + hf) * P
                                nc.sync.dma_start(
                                    out=w2t[pr][:, hf, :],
                                    in_=w2[l, ro:ro + P,
                                           nslab * 512:(nslab + 1) * 512])
                    else:
                        w2t = [w2pool.tile([P, 512], bf16, tag="w2slab", name="w2slab")
                               for _ in range(16)]
                        for k2 in range(16):
                            ro = c * 2048 + k2 * P
                            nc.sync.dma_start(
                                out=w2t[k2],
                                in_=w2[l, ro:ro + P,
                                       nslab * 512:(nslab + 1) * 512])
                    for ni_ in range(4):
                        nt = nslab * 4 + ni_
                        sc_ap = (pt[:, PC_B2 + nt:PC_B2 + nt + 1]
                                 if c == 1 else 0.0)
                        for s in range(BPC):
                            off, T = SOFF[s], STOK[s]
                            ps = psp.tile([P, 512], f32, tag="mm", bufs=3, name="psmm")
                            if FP8:
                                for pr in range(8):
                                    nc.tensor.matmul(
                                        ps[:, :T],
                                        w2t[pr][:, :, ni_ * P:(ni_ + 1) * P],
                                        gp2[pr][:, :, off:off + T],
                                        start=(pr == 0), stop=(pr == 7),
                                        perf_mode=DR)
                                t_ = tpool.tile([P, 512], f32, tag="deq", name="deq")
                                nc.vector.tensor_scalar(
                                    out=t_[:, :T], in0=ps[:, :T], scalar1=DQ,
                                    scalar2=sc_ap, op0=OP.mult, op1=OP.add)
                                nc.vector.tensor_add(
                                    xt[nt][:, off:off + T],
                                    xt[nt][:, off:off + T], t_[:, :T])
                            else:
                                for k2 in range(16):
                                    nc.tensor.matmul(
                                        ps[:, :T], w2t[k2][:, ni_ * P:(ni_ + 1) * P],
                                        gt[k2][:, off:off + T],
                                        start=(k2 == 0), stop=(k2 == 15))
                                nc.vector.scalar_tensor_tensor(
                                    out=xt[nt][:, off:off + T], in0=ps[:, :T],
                                    scalar=sc_ap,
                                    in1=xt[nt][:, off:off + T],
                                    op0=OP.add, op1=OP.add)

        # ---------------- epilogue: ln_post + out proj ----------------
        gp = [postt[:, k:k + 1] for k in range(KT)]
        bp = [postt[:, 8 + k:8 + k + 1] for k in range(KT)]
        for s in range(BPC):
            hE = [ypool.tile([P, M], bf16, tag=f"hE{k}", name=f"hE{k}")
                  for k in range(KT)]
            hsl = lambda k, lo, hi: hE[k][:, lo - SOFF[s]:hi - SOFF[s]]
            emit_ln(SOFF[s], M, gp, bp, hsl, SOFF[s])
            pso = psp.tile([VAE, M], f32, tag="row", bufs=2, name="psout")
            for k in range(KT):
                nc.tensor.matmul(pso, woutt[:, k * VAE:(k + 1) * VAE],
                                 hE[k][:, :M],
                                 start=(k == 0), stop=(k == KT - 1))
            y = ypool.tile([VAE, M], f32, tag="y", name="y")
            nc.vector.tensor_scalar_add(y, pso, boutt)
            nc.sync.dma_start(out=out_d[s].rearrange("r c -> c r"), in_=y)

    nc.finalize()
    return nc


def kernel(**inputs):
    global _PROG, _PROG_KEY
    from concourse.bass_utils import run_bass_kernel_spmd
    in_maps, order, TA, TB = _host_prep(inputs)
    zb = not np.any(inputs["qkv_b"][:, 2 * D:])
    if _PROG is None or _PROG_KEY != (TA, TB, zb):
        _PROG = _build_bass(TA, TB, zb)
        _PROG_KEY = (TA, TB, zb)
    res = run_bass_kernel_spmd(_PROG, in_maps, list(range(NCORES)))
    out = np.zeros((B, M, VAE), np.float32)
    for c in range(NCORES):
        o = res.results[c]["out"]
        out[int(order[c])] = o[0]
        out[int(order[NCORES + c])] = o[1]
    return out.astype(np.float32)


# revision 15
# speedup vs baseline: 1.0392x; 1.0029x over previous
"""Trainium2 Bass kernel for nn_Decoder_1D_Matryoshka (12-layer masked decoder).

v2: exploits the Matryoshka sparsity. Rows >= 256+Ni are dead (never attended
by any row that reaches the output), so each sample only needs T_i = 256+Ni
tokens. Samples are split into two shape classes (TA = 8 largest, TB = rest),
one of each per core; per-sample masks/zero-padding carry correctness so the
SPMD program is identical across cores. Attention exploits the block mask
structure: mask-token keys are only attended by mask-token queries, latent
keys only by causally-later latent queries + all mask queries. The Ni cutoff
is a per-key-partition exp bias; only the 128x128 causal diagonal needs a
tensor mask add (static triangle). Softmax denominators come free from the
AV matmul via an interleaved ones column in V. Weight DMAs are hoisted so
each layer's weights stream once (not once per sample).

Residual stream kept transposed (features on partitions, tokens free) as in
v1; attention uses transposed scores S^T = K Q^T.
"""

import os
import numpy as np
import ml_dtypes

B, NLAT, DIN = 16, 256, 32
D, H, NL, VAE = 1024, 16, 12, 16
M = 256
DH = 64
NCORES = 8
BPC = 2
P = 128
KT = D // P       # 8 k-bands over D
FF = 4 * D
NL_OVR = int(os.environ.get("BASSK_NL", NL))

BF16 = ml_dtypes.bfloat16
FP8NP = ml_dtypes.float8_e4m3
FP8 = os.environ.get("BASSK_FP8", "0") == "1"
WS = 64.0         # fp8 weight scale

# param-tile column layout (one [128, 104] f32 DMA per layer)
PC_BQKV = 0       # 24 cols: qkv bias, n-tile t at col t (q cols pre-scaled)
PC_BPROJ = 24     # 8
PC_B1 = 32        # 32
PC_B2 = 64        # 8
PC_G1 = 72        # 8
PC_BL1 = 80       # 8
PC_G2 = 88        # 8
PC_BL2 = 96       # 8
PCOLS = 104


def _layernorm_np(x, g, b, eps=1e-5):
    mu = x.mean(-1, keepdims=True)
    var = ((x - mu) ** 2).mean(-1, keepdims=True)
    return (x - mu) / np.sqrt(var + eps) * g + b


def _classes(ni):
    T = 256 + np.asarray(ni).astype(np.int64)
    Tp = ((T + 127) // 128) * 128
    order = np.argsort(-Tp, kind="stable")
    TA = int(Tp[order[0]])
    TB = int(Tp[order[NCORES]])
    return order, TA, TB


def _host_prep(inputs):
    """Returns (per_core_in_maps, order, TA, TB)."""
    f32 = np.float32
    ni = np.asarray(inputs["num_activated"]).astype(np.int64)
    order, TA, TB = _classes(ni)
    CT = TA + TB

    lat = inputs["latents"].astype(f32)
    x_lat = lat.reshape(B * NLAT, DIN) @ inputs["input_w"].astype(f32)
    x_lat = x_lat.reshape(B, NLAT, D) + inputs["input_b"][None, None, :]
    x_lat = x_lat + inputs["latents_pos_embed"][None, :, :]
    mt = inputs["mask_tokens"].reshape(1, 1, D) + inputs["pos_embed_full"]
    mt = np.broadcast_to(mt, (B, M, D))
    x = np.concatenate([mt, x_lat], axis=1)                       # (B, L, D)
    x = _layernorm_np(x, inputs["ln_pre_g"], inputs["ln_pre_b"]).astype(f32)

    # static causal triangle for the diagonal 128x128 latent blocks
    # tri[key, query] = -30 where key > query
    rr = np.arange(P)
    tri = np.where(rr[:, None] <= rr[None, :], f32(0.0), f32(-30.0))

    # ---- static (identical on every core) weights, repacked ----
    scale = np.float32(DH ** -0.5)
    wqkv = inputs["qkv_w"].astype(f32).copy()                      # (NL,D,3D)
    wqkv[:, :, :D] *= scale
    bqkv = inputs["qkv_b"].astype(f32).copy()                      # (NL,3D)
    bqkv[:, :D] *= scale

    params = np.zeros((NL, P, PCOLS), f32)
    params[:, :, PC_BQKV:PC_BQKV + 24] = bqkv.reshape(NL, 24, P).transpose(0, 2, 1)
    params[:, :, PC_BPROJ:PC_BPROJ + 8] = inputs["proj_b"].reshape(NL, 8, P).transpose(0, 2, 1)
    params[:, :, PC_B1:PC_B1 + 32] = inputs["fc1_b"].reshape(NL, 32, P).transpose(0, 2, 1)
    params[:, :, PC_B2:PC_B2 + 8] = inputs["fc2_b"].reshape(NL, 8, P).transpose(0, 2, 1)
    params[:, :, PC_G1:PC_G1 + 8] = inputs["ln1_g"].reshape(NL, 8, P).transpose(0, 2, 1)
    params[:, :, PC_BL1:PC_BL1 + 8] = inputs["ln1_b"].reshape(NL, 8, P).transpose(0, 2, 1)
    params[:, :, PC_G2:PC_G2 + 8] = inputs["ln2_g"].reshape(NL, 8, P).transpose(0, 2, 1)
    params[:, :, PC_BL2:PC_BL2 + 8] = inputs["ln2_b"].reshape(NL, 8, P).transpose(0, 2, 1)

    # v-bias rows, seeded into psum via K=1 ones-matmul: (NL, 1, D)
    rowparams = bqkv[:, 2 * D:3 * D].reshape(NL, 1, D).astype(f32)

    postparams = np.zeros((P, 16), f32)
    postparams[:, 0:8] = inputs["ln_post_g"].reshape(8, P).T
    postparams[:, 8:16] = inputs["ln_post_b"].reshape(8, P).T

    if FP8:
        def q8(w):
            return np.ascontiguousarray(
                np.clip(w.astype(f32) * WS, -240, 240).astype(FP8NP))
        wmain = {
            "wqkv": q8(wqkv),
            "wproj": q8(inputs["proj_w"]),
            "w1": q8(inputs["fc1_w"]),
            "w2": q8(inputs["fc2_w"]),
        }
        rowparams = rowparams * np.float32(WS)
    else:
        wmain = {
            "wqkv": np.ascontiguousarray(wqkv.astype(BF16)),
            "wproj": np.ascontiguousarray(inputs["proj_w"].astype(BF16)),
            "w1": np.ascontiguousarray(inputs["fc1_w"].astype(BF16)),
            "w2": np.ascontiguousarray(inputs["fc2_w"].astype(BF16)),
        }
    static = {
        **wmain,
        "params": params,
        "rowparams": np.ascontiguousarray(rowparams.astype(BF16)),
        "postparams": postparams,
        "tri": np.ascontiguousarray(tri.astype(BF16)),
        "wout": np.ascontiguousarray(
            inputs["out_w"].astype(BF16).reshape(KT, P, VAE)),
        "bout": inputs["out_b"].astype(f32).reshape(VAE, 1),
    }

    in_maps = []
    for c in range(NCORES):
        sA, sB = int(order[c]), int(order[NCORES + c])
        x0T = np.zeros((D, CT), f32)
        x0T[:, :256 + ni[sA]] = x[sA, :256 + ni[sA]].T
        x0T[:, TA:TA + 256 + ni[sB]] = x[sB, :256 + ni[sB]].T
        # per-latent-tile cutoff bias: cols 0..1 sample A tiles, 2..3 sample B
        latcut = np.full((P, 4), f32(-30.0))
        for j in range((TA - 256) // P):
            latcut[:, j] = np.where(j * P + rr < ni[sA], f32(0.0), f32(-30.0))
        for j in range((TB - 256) // P):
            latcut[:, 2 + j] = np.where(j * P + rr < ni[sB], f32(0.0), f32(-30.0))
        m = {"x0T": x0T, "latcut": latcut}
        m.update(static)
        in_maps.append(m)
    return in_maps, order, TA, TB


_PROG = None
_PROG_KEY = None


def _build_bass(TA, TB, zb):
    import concourse.bass as bass
    import concourse.bacc as bacc
    import concourse.tile as tile
    import concourse.mybir as mybir
    from contextlib import ExitStack

    f32 = mybir.dt.float32
    bf16 = mybir.dt.bfloat16
    f8 = mybir.dt.float8e4
    wdt = f8 if FP8 else bf16
    DR = mybir.MatmulPerfMode.DoubleRow if FP8 else None
    DQ = 1.0 / WS
    AF = mybir.ActivationFunctionType
    OP = mybir.AluOpType
    NLn = NL_OVR
    CT = TA + TB
    JA = TA // P          # key tiles sample A (4 when TA=512)
    JB = TB // P
    SOFF = (0, TA)
    STOK = (TA, TB)
    SJ = (JA, JB)
    SLC = (0, 2)          # latcut col base per sample

    nc = bacc.Bacc(None, target_bir_lowering=False, debug=False)
    dp = nc.declare_dram_parameter
    x0T = dp("x0T", [D, CT], f32, isOutput=False)
    latcut_d = dp("latcut", [P, 4], f32, isOutput=False)
    tri_d = dp("tri", [P, P], bf16, isOutput=False)
    wqkv = dp("wqkv", [NL, D, 3 * D], wdt, isOutput=False)
    wproj = dp("wproj", [NL, D, D], wdt, isOutput=False)
    w1 = dp("w1", [NL, D, FF], wdt, isOutput=False)
    w2 = dp("w2", [NL, FF, D], wdt, isOutput=False)
    params = dp("params", [NL, P, PCOLS], f32, isOutput=False)
    rowparams = dp("rowparams", [NL, 1, D], bf16, isOutput=False)
    postparams = dp("postparams", [P, 16], f32, isOutput=False)
    wout_d = dp("wout", [KT, P, VAE], bf16, isOutput=False)
    bout_d = dp("bout", [VAE, 1], f32, isOutput=False)
    out_d = dp("out", [BPC, M, VAE], f32, isOutput=True)

    with ExitStack() as ctx:
        tc = ctx.enter_context(tile.TileContext(nc))
        # ---- pools ----
        persist = ctx.enter_context(tc.tile_pool(name="persist", bufs=1))
        wpool = ctx.enter_context(tc.tile_pool(name="wslab", bufs=16))
        w2pool = ctx.enter_context(tc.tile_pool(name="w2slab", bufs=18))
        hpool = ctx.enter_context(tc.tile_pool(name="h", bufs=1))
        qkpool = ctx.enter_context(tc.tile_pool(name="qk", bufs=1))
        apool = ctx.enter_context(tc.tile_pool(name="attnT", bufs=1))
        gpool = ctx.enter_context(tc.tile_pool(name="g", bufs=1))
        epool = ctx.enter_context(tc.tile_pool(name="expS", bufs=2))
        tpool = ctx.enter_context(tc.tile_pool(name="scratch", bufs=2))
        spool = ctx.enter_context(tc.tile_pool(name="stats", bufs=5))
        rpool = ctx.enter_context(tc.tile_pool(name="rsb", bufs=2))
        bpool = ctx.enter_context(tc.tile_pool(name="lnb", bufs=1))
        ppool = ctx.enter_context(tc.tile_pool(name="lparams", bufs=2))
        ypool = ctx.enter_context(tc.tile_pool(name="yout", bufs=1))
        psp = ctx.enter_context(tc.tile_pool(name="ps", bufs=1, space="PSUM"))

        # ---- persistent tiles ----
        xt = [persist.tile([P, CT], f32, tag=f"x{k}", name=f"x{k}") for k in range(KT)]
        ones_f = persist.tile([P, 1], f32, tag="ones_f", name="ones_f")
        ones_b = persist.tile([P, 1], bf16, tag="ones_b", name="ones_b")
        ones_row = persist.tile([1, P], bf16, tag="ones_row", name="ones_row")
        trit = persist.tile([P, P], bf16, tag="tri", name="tri")
        latc = persist.tile([P, 4], f32, tag="latc", name="latc")
        # V with interleaved ones column per head: [P, J, 16 heads, 65]
        vtt = [persist.tile([P, SJ[s], H, DH + 1], bf16, tag=f"vt{s}", name=f"vt{s}")
               for s in range(BPC)]
        woutt = persist.tile([P, KT * VAE], bf16, tag="wo", name="wo")
        boutt = persist.tile([VAE, 1], f32, tag="bout", name="bout")
        postt = persist.tile([P, 16], f32, tag="post", name="post")
        epst = persist.tile([1, 1], f32, tag="eps", name="eps")
        nc.vector.memset(epst, 1e-5)
        zerot = persist.tile([P, 1], f32, tag="zerot", name="zerot")
        nc.vector.memset(zerot, 0.0)

        for k in range(KT):
            nc.sync.dma_start(out=xt[k], in_=x0T[k * P:(k + 1) * P, :])
            nc.sync.dma_start(out=woutt[:, k * VAE:(k + 1) * VAE], in_=wout_d[k])
        nc.vector.memset(ones_f, 1.0)
        nc.vector.memset(ones_b, 1.0)
        nc.vector.memset(ones_row, 1.0)
        for s in range(BPC):
            nc.vector.memset(vtt[s], 1.0)
        nc.sync.dma_start(out=trit, in_=tri_d[:, :])
        nc.sync.dma_start(out=latc, in_=latcut_d[:, :])
        nc.sync.dma_start(out=boutt, in_=bout_d[:, :])
        nc.sync.dma_start(out=postt, in_=postparams[:, :])

        def emit_ln(src_off, width, g_ap, b_ap, hsl, h_off, xq=None):
            """LN over features (partitions) of x[:, src_off:src_off+width] ->
            hsl(k, h_off, h_off+width). xq: optional precomputed full-width
            (xb, xsq) bf16 tiles."""
            psx = psp.tile([1, 512], f32, tag="row", bufs=2, name="psx")
            psxx = psp.tile([1, 512], f32, tag="row", bufs=2, name="psxx")
            for k in range(KT):
                xs = xt[k][:, src_off:src_off + width]
                if xq is None:
                    xsq = tpool.tile([P, 512], bf16, tag="xsq", name="xsq")
                    xb = tpool.tile([P, 512], bf16, tag="xb", name="xb")
                    nc.scalar.activation(xsq[:, :width], xs, AF.Square)
                    nc.vector.tensor_copy(out=xb[:, :width], in_=xs)
                    xb_ap = xb[:, :width]
                    xsq_ap = xsq[:, :width]
                else:
                    xb_ap = xq[0][k][:, src_off:src_off + width]
                    xsq_ap = xq[1][k][:, src_off:src_off + width]
                nc.tensor.matmul(psx[:, :width], ones_b, xb_ap,
                                 start=(k == 0), stop=(k == KT - 1))
                nc.tensor.matmul(psxx[:, :width], ones_b, xsq_ap,
                                 start=(k == 0), stop=(k == KT - 1))
            mu = spool.tile([1, 512], f32, tag="stat", name="mu")
            e2 = spool.tile([1, 512], f32, tag="stat", name="e2")
            rstd = spool.tile([1, 512], f32, tag="stat", name="rstd")
            nc.vector.tensor_scalar_mul(mu[:, :width], psx[:, :width], 1.0 / D)
            nc.vector.tensor_scalar_mul(e2[:, :width], psxx[:, :width], 1.0 / D)
            musq = spool.tile([1, 512], f32, tag="stat", name="musq")
            nc.vector.tensor_mul(musq[:, :width], mu[:, :width], mu[:, :width])
            nc.vector.tensor_sub(e2[:, :width], e2[:, :width], musq[:, :width])
            nc.scalar.activation(e2[:, :width], e2[:, :width], AF.Sqrt,
                                 bias=epst)
            nc.vector.reciprocal(rstd[:, :width], e2[:, :width])
            pmu = bpool.tile([P, 512], f32, tag="pmu", name="pmu")
            prs = bpool.tile([P, 512], f32, tag="prs", name="prs")
            nc.gpsimd.partition_broadcast(pmu[:, :width], mu[:, :width])
            nc.gpsimd.partition_broadcast(prs[:, :width], rstd[:, :width])
            for k in range(KT):
                xs = xt[k][:, src_off:src_off + width]
                t = tpool.tile([P, 512], f32, tag="lnt", name="lnt")
                nc.vector.tensor_sub(t[:, :width], xs, pmu[:, :width])
                nc.vector.tensor_mul(t[:, :width], t[:, :width], prs[:, :width])
                nc.vector.tensor_scalar(
                    out=hsl(k, h_off, h_off + width), in0=t[:, :width],
                    scalar1=g_ap[k], scalar2=b_ap[k],
                    op0=OP.mult, op1=OP.add)

        for l in range(NLn):
            pt = ppool.tile([P, PCOLS], f32, tag="pt", name="pt")
            rowp = ppool.tile([1, D], bf16, tag="rowp", name="rowp")
            nc.sync.dma_start(out=pt, in_=params[l])
            nc.sync.dma_start(out=rowp, in_=rowparams[l])
            g1 = [pt[:, PC_G1 + k:PC_G1 + k + 1] for k in range(KT)]
            bl1 = [pt[:, PC_BL1 + k:PC_BL1 + k + 1] for k in range(KT)]
            g2 = [pt[:, PC_G2 + k:PC_G2 + k + 1] for k in range(KT)]
            bl2 = [pt[:, PC_BL2 + k:PC_BL2 + k + 1] for k in range(KT)]

            # ---------------- LN1 -> h (both samples) ----------------
            if FP8:
                hp = [hpool.tile([P, 2, CT], f8, tag=f"h{i}", name=f"h{i}")
                      for i in range(4)]
                hsl = lambda k, lo, hi: hp[k // 2][:, k % 2, lo:hi]
            else:
                htiles = [hpool.tile([P, CT], bf16, tag=f"h{k}", name=f"h{k}") for k in range(KT)]
                hsl = lambda k, lo, hi: htiles[k][:, lo:hi]
            for s in range(BPC):
                emit_ln(SOFF[s], STOK[s], g1, bl1, hsl, SOFF[s])

            def load_wpair(src_d, cl, ch):
                wts = [wpool.tile([P, 2, ch - cl], f8, tag="wslab",
                                  name="wslab") for _ in range(4)]
                for pr in range(4):
                    for hf in range(2):
                        nc.sync.dma_start(
                            out=wts[pr][:, hf, :],
                            in_=src_d[l, (2 * pr + hf) * P:(2 * pr + hf + 1) * P,
                                      cl:ch])
                return wts

            def chainA(ps_ap, wts, col, off, T, seed=None):
                if seed is not None:
                    nc.tensor.matmul(ps_ap, ones_row, seed,
                                     start=True, stop=False)
                if FP8:
                    for pr in range(4):
                        nc.tensor.matmul(
                            ps_ap, wts[pr][:, :, col:col + P],
                            hp[pr][:, :, off:off + T],
                            start=(pr == 0 and seed is None), stop=(pr == 3),
                            perf_mode=DR)
                else:
                    for k in range(KT):
                        nc.tensor.matmul(
                            ps_ap, wts[k][:, col:col + P],
                            htiles[k][:, off:off + T],
                            start=(k == 0 and seed is None), stop=(k == KT - 1))

            # ---------------- QKV/attn/proj with cross-sample overlap ----
            qt = [qkpool.tile([P, CT], bf16, tag=f"q{n}", name=f"q{n}") for n in range(KT)]
            kt_ = [qkpool.tile([P, CT], bf16, tag=f"k{n}", name=f"k{n}") for n in range(KT)]
            if FP8:
                atp = [apool.tile([P, 2, CT], f8, tag=f"at{i}", name=f"at{i}")
                       for i in range(4)]
                asl = lambda k, po, lo, hi: atp[k // 2][po:po + DH, k % 2, lo:hi]
            else:
                attnT = [apool.tile([P, CT], bf16, tag=f"at{k}", name=f"at{k}") for k in range(KT)]
                asl = lambda k, po, lo, hi: attnT[k][po:po + DH, lo:hi]

            def qkv_units(s):
                off, T = SOFF[s], STOK[s]
                units = []
                wstore = {}
                for slab in range(4):      # cols 0..2048 of wqkv: q then k
                    def load(slab=slab):
                        if FP8:
                            wstore[slab] = load_wpair(
                                wqkv, slab * 512, (slab + 1) * 512)
                        else:
                            wts = [wpool.tile([P, 512], bf16, tag="wslab",
                                              name="wslab") for _ in range(KT)]
                            for k in range(KT):
                                nc.sync.dma_start(
                                    out=wts[k],
                                    in_=wqkv[l, k * P:(k + 1) * P,
                                             slab * 512:(slab + 1) * 512])
                            wstore[slab] = wts
                    units.append(load)
                    for ni_ in range(4):
                        def chain(slab=slab, ni_=ni_):
                            nt = slab * 4 + ni_
                            dst = qt[nt] if nt < KT else kt_[nt - KT]
                            ps = psp.tile([P, 512], f32, tag="mm", bufs=3,
                                          name="psmm")
                            chainA(ps[:, :T], wstore[slab], ni_ * P, off, T)
                            if FP8:
                                nc.vector.tensor_scalar(
                                    out=dst[:, off:off + T], in0=ps[:, :T],
                                    scalar1=DQ,
                                    scalar2=pt[:, PC_BQKV + nt:PC_BQKV + nt + 1],
                                    op0=OP.mult, op1=OP.add)
                            else:
                                nc.vector.tensor_scalar_add(
                                    dst[:, off:off + T], ps[:, :T],
                                    pt[:, PC_BQKV + nt:PC_BQKV + nt + 1])
                        units.append(chain)
                for slab in range(4, 6):   # v cols 2048..3072
                    def load(slab=slab):
                        if FP8:
                            wstore[slab] = load_wpair(
                                wqkv, slab * 512, (slab + 1) * 512)
                        else:
                            wts = [wpool.tile([P, 512], bf16, tag="wslab",
                                              name="wslab") for _ in range(KT)]
                            for k in range(KT):
                                nc.sync.dma_start(
                                    out=wts[k],
                                    in_=wqkv[l, k * P:(k + 1) * P,
                                             slab * 512:(slab + 1) * 512])
                            wstore[slab] = wts
                    units.append(load)
                    for mb in range(SJ[s]):
                        def vchain(slab=slab, mb=mb):
                            nh = slab - 4
                            wts = wstore[slab]
                            ps = psp.tile([P, 8, DH], f32, tag="mm", bufs=3,
                                          name="psmm")
                            if not zb:
                                nc.tensor.matmul(
                                    ps, ones_row,
                                    rowp[:, nh * 512:(nh + 1) * 512],
                                    start=True, stop=False)
                            if FP8:
                                for pr in range(4):
                                    nc.tensor.matmul(
                                        ps, hp[pr][:, :, off + mb * P:off + (mb + 1) * P],
                                        wts[pr][:, :, :],
                                        start=(zb and pr == 0), stop=(pr == 3),
                                        perf_mode=DR)
                                nc.vector.tensor_scalar_mul(
                                    vtt[s][:, mb, nh * 8:(nh + 1) * 8, 0:DH],
                                    ps, DQ)
                            else:
                                for k in range(KT):
                                    nc.tensor.matmul(
                                        ps, htiles[k][:, off + mb * P:off + (mb + 1) * P],
                                        wts[k], start=(zb and k == 0),
                                        stop=(k == KT - 1))
                                nc.vector.tensor_copy(
                                    out=vtt[s][:, mb, nh * 8:(nh + 1) * 8, 0:DH],
                                    in_=ps)
                        units.append(vchain)
                return units

            pwstore = {}

            def proj_units():
                units = []
                for slab in range(2):
                    def load(slab=slab):
                        if FP8:
                            pwstore[slab] = load_wpair(
                                wproj, slab * 512, (slab + 1) * 512)
                        else:
                            wts = [wpool.tile([P, 512], bf16, tag="wslab",
                                              name="wslab") for _ in range(KT)]
                            for k in range(KT):
                                nc.sync.dma_start(
                                    out=wts[k],
                                    in_=wproj[l, k * P:(k + 1) * P,
                                              slab * 512:(slab + 1) * 512])
                            pwstore[slab] = wts
                    units.append(load)
                return units

            def proj_chain(s, slab, ni_):
                off, T = SOFF[s], STOK[s]
                nt = slab * 4 + ni_
                wts = pwstore[slab]
                ps = psp.tile([P, 512], f32, tag="mm", bufs=3, name="psmm")
                if FP8:
                    for pr in range(4):
                        nc.tensor.matmul(
                            ps[:, :T], wts[pr][:, :, ni_ * P:(ni_ + 1) * P],
                            atp[pr][:, :, off:off + T],
                            start=(pr == 0), stop=(pr == 3),
                            perf_mode=DR)
                    t_ = tpool.tile([P, 512], f32, tag="deq", name="deq")
                    nc.vector.tensor_scalar(
                        out=t_[:, :T], in0=ps[:, :T], scalar1=DQ,
                        scalar2=pt[:, PC_BPROJ + nt:PC_BPROJ + nt + 1],
                        op0=OP.mult, op1=OP.add)
                    nc.vector.tensor_add(
                        xt[nt][:, off:off + T], xt[nt][:, off:off + T],
                        t_[:, :T])
                else:
                    for k in range(KT):
                        nc.tensor.matmul(
                            ps[:, :T], wts[k][:, ni_ * P:(ni_ + 1) * P],
                            attnT[k][:, off:off + T],
                            start=(k == 0), stop=(k == KT - 1))
                    nc.vector.scalar_tensor_tensor(
                        out=xt[nt][:, off:off + T], in0=ps[:, :T],
                        scalar=pt[:, PC_BPROJ + nt:PC_BPROJ + nt + 1],
                        in1=xt[nt][:, off:off + T],
                        op0=OP.add, op1=OP.add)

            def attn_head(s, hh):
                off, T, J = SOFF[s], STOK[s], SJ[s]
                band = hh // 2
                po = (hh % 2) * DH
                q_ap = qt[band][po:po + DH, off:off + T]
                k_ap = kt_[band][po:po + DH, :]
                etiles = []
                for j in range(J):
                    e = epool.tile([P, 256 if j < 2 else 512], bf16,
                                   tag=f"e{j}", bufs=2, name=f"e{j}")
                    ks = k_ap[:, off + j * P:off + (j + 1) * P]
                    psS = psp.tile([P, 512], f32, tag="mm", bufs=3,
                                   name="psS")
                    if j < 2:
                        nc.tensor.matmul(psS[:, :M], ks, q_ap[:, :M],
                                         start=True, stop=True)
                        nc.scalar.activation(e[:, :M], psS[:, :M], AF.Exp,
                                             bias=zerot)
                    else:
                        lc = latc[:, SLC[s] + (j - 2):SLC[s] + (j - 2) + 1]
                        dq = j * P
                        nc.tensor.matmul(psS[:, :T], ks, q_ap,
                                         start=True, stop=True)
                        nc.vector.tensor_scalar_add(
                            psS[:, :M], psS[:, :M], lc)
                        nc.vector.tensor_add(
                            psS[:, dq:dq + P], psS[:, dq:dq + P], trit)
                        nc.scalar.activation(e[:, :T], psS[:, :T],
                                             AF.Exp, bias=zerot)
                    etiles.append(e)
                rs = spool.tile([1, 512], f32, tag="stat", name="rs")
                rsb = rpool.tile([DH, 512], f32, tag="rsb", name="rsb")
                psA = psp.tile([DH + 1, M], f32, tag="OTa", bufs=2, name="psA")
                for j in range(J):
                    nc.tensor.matmul(psA, vtt[s][:, j, hh, :],
                                     etiles[j][:, :M],
                                     start=(j == 0), stop=(j == J - 1))
                nc.vector.reciprocal(rs[:, :M], psA[DH:DH + 1, :])
                nc.gpsimd.partition_broadcast(rsb[:, :M], rs[:, :M])
                nc.vector.tensor_mul(asl(band, po, off, off + M),
                                     psA[0:DH, :], rsb[:, :M])
                for cch in range(J - 2):
                    qs = M + cch * P
                    pc = psp.tile([DH + 1, P], f32, tag="OTc", bufs=1,
                                  name=f"psC{cch}")
                    for j in range(2, 3 + cch):
                        nc.tensor.matmul(pc, vtt[s][:, j, hh, :],
                                         etiles[j][:, qs:qs + P],
                                         start=(j == 2), stop=(j == 2 + cch))
                    nc.vector.reciprocal(rs[:, qs:qs + P],
                                         pc[DH:DH + 1, :])
                    nc.gpsimd.partition_broadcast(rsb[:, qs:qs + P],
                                                  rs[:, qs:qs + P])
                    nc.vector.tensor_mul(asl(band, po, off + qs, off + qs + P),
                                         pc[0:DH, :], rsb[:, qs:qs + P])

            # emit: QKV(A); attn(A) fed by QKV(B) units; attn(B) fed by
            # proj loads + proj(A) chains; then proj(B).
            for u in qkv_units(0):
                u()
            fill = qkv_units(1)
            fi_ = 0
            for hh in range(H):
                take = (len(fill) * (hh + 1)) // H
                while fi_ < take:
                    fill[fi_](); fi_ += 1
                attn_head(0, hh)
            while fi_ < len(fill):
                fill[fi_](); fi_ += 1
            pfill = proj_units() + [
                (lambda slab=slab, ni_=ni_: proj_chain(0, slab, ni_))
                for slab in range(2) for ni_ in range(4)]
            fi_ = 0
            for hh in range(H):
                take = (len(pfill) * (hh + 1)) // H
                while fi_ < take:
                    pfill[fi_](); fi_ += 1
                attn_head(1, hh)
            while fi_ < len(pfill):
                pfill[fi_](); fi_ += 1

            # ---------------- LN2 -> h (A early, overlapped w/ proj B) ---
            if FP8:
                hp = [hpool.tile([P, 2, CT], f8, tag=f"h{i}", name=f"h{i}")
                      for i in range(4)]
                hsl = lambda k, lo, hi: hp[k // 2][:, k % 2, lo:hi]
            else:
                htiles = [hpool.tile([P, CT], bf16, tag=f"h{k}", name=f"h{k}") for k in range(KT)]
                hsl = lambda k, lo, hi: htiles[k][:, lo:hi]
            emit_ln(SOFF[0], STOK[0], g2, bl2, hsl, SOFF[0])
            for slab in range(2):
                for ni_ in range(4):
                    proj_chain(1, slab, ni_)
            emit_ln(SOFF[1], STOK[1], g2, bl2, hsl, SOFF[1])

            # ---------------- FF, hoisted ----------------
            for c in range(2):
                if FP8:
                    gp2 = [gpool.tile([P, 2, CT], f8, tag=f"g{i}", name=f"g{i}")
                           for i in range(8)]
                    gsl = lambda fi, lo, hi: gp2[fi // 2][:, fi % 2, lo:hi]
                else:
                    gt = [gpool.tile([P, CT], bf16, tag=f"g{i}", name=f"g{i}") for i in range(16)]
                    gsl = lambda fi, lo, hi: gt[fi][:, lo:hi]
                for slab in range(4):
                    co = c * 2048 + slab * 512
                    if FP8:
                        wts = load_wpair(w1, co, co + 512)
                    else:
                        wts = [wpool.tile([P, 512], bf16, tag="wslab", name="wslab")
                               for _ in range(KT)]
                        for k in range(KT):
                            nc.sync.dma_start(
                                out=wts[k],
                                in_=w1[l, k * P:(k + 1) * P, co:co + 512])
                    for ni_ in range(4):
                        fi = slab * 4 + ni_           # within chunk c
                        bc = PC_B1 + c * 16 + fi
                        for s in range(BPC):
                            off, T = SOFF[s], STOK[s]
                            ps = psp.tile([P, 512], f32, tag="mm", bufs=3, name="psmm")
                            chainA(ps[:, :T], wts, ni_ * P, off, T)
                            nc.scalar.activation(gsl(fi, off, off + T),
                                                 ps[:, :T], AF.Gelu,
                                                 bias=pt[:, bc:bc + 1],
                                                 scale=(DQ if FP8 else 1.0))
                for nslab in range(2):
                    if FP8:
                        w2t = [w2pool.tile([P, 2, 512], f8, tag="w2slab",
                                           name="w2slab") for _ in range(8)]
                        for pr in range(8):
                            for hf in range(2):
                                ro = c * 2048 + (2 * pr + hf) * P
                                nc.sync.dma_start(
                                    out=w2t[pr][:, hf, :],
                                    in_=w2[l, ro:ro + P,
                                           nslab * 512:(nslab + 1) * 512])
                    else:
                        w2t = [w2pool.tile([P, 512], bf16, tag="w2slab", name="w2slab")
                               for _ in range(16)]
                        for k2 in range(16):
                            ro = c * 2048 + k2 * P
                            nc.sync.dma_start(
                                out=w2t[k2],
                                in_=w2[l, ro:ro + P,
                                       nslab * 512:(nslab + 1) * 512])
                    for ni_ in range(4):
                        nt = nslab * 4 + ni_
                        sc_ap = (pt[:, PC_B2 + nt:PC_B2 + nt + 1]
                                 if c == 1 else 0.0)
                        for s in range(BPC):
                            off, T = SOFF[s], STOK[s]
                            ps = psp.tile([P, 512], f32, tag="mm", bufs=3, name="psmm")
                            if FP8:
                                for pr in range(8):
                                    nc.tensor.matmul(
                                        ps[:, :T],
                                        w2t[pr][:, :, ni_ * P:(ni_ + 1) * P],
                                        gp2[pr][:, :, off:off + T],
                                        start=(pr == 0), stop=(pr == 7),
                                        perf_mode=DR)
                                t_ = tpool.tile([P, 512], f32, tag="deq", name="deq")
                                nc.vector.tensor_scalar(
                                    out=t_[:, :T], in0=ps[:, :T], scalar1=DQ,
                                    scalar2=sc_ap, op0=OP.mult, op1=OP.add)
                                nc.vector.tensor_add(
                                    xt[nt][:, off:off + T],
                                    xt[nt][:, off:off + T], t_[:, :T])
                            else:
                                for k2 in range(16):
                                    nc.tensor.matmul(
                                        ps[:, :T], w2t[k2][:, ni_ * P:(ni_ + 1) * P],
                                        gt[k2][:, off:off + T],
                                        start=(k2 == 0), stop=(k2 == 15))
                                nc.vector.scalar_tensor_tensor(
                                    out=xt[nt][:, off:off + T], in0=ps[:, :T],
                                    scalar=sc_ap,
                                    in1=xt[nt][:, off:off + T],
                                    op0=OP.add, op1=OP.add)

        # ---------------- epilogue: ln_post + out proj ----------------
        gp = [postt[:, k:k + 1] for k in range(KT)]
        bp = [postt[:, 8 + k:8 + k + 1] for k in range(KT)]
        for s in range(BPC):
            hE = [ypool.tile([P, M], bf16, tag=f"hE{k}", name=f"hE{k}")
                  for k in range(KT)]
            hsl = lambda k, lo, hi: hE[k][:, lo - SOFF[s]:hi - SOFF[s]]
            emit_ln(SOFF[s], M, gp, bp, hsl, SOFF[s])
            pso = psp.tile([VAE, M], f32, tag="row", bufs=2, name="psout")
            for k in range(KT):
                nc.tensor.matmul(pso, woutt[:, k * VAE:(k + 1) * VAE],
                                 hE[k][:, :M],
                                 start=(k == 0), stop=(k == KT - 1))
            y = ypool.tile([VAE, M], f32, tag="y", name="y")
            nc.vector.tensor_scalar_add(y, pso, boutt)
            nc.sync.dma_start(out=out_d[s].rearrange("r c -> c r"), in_=y)

    nc.finalize()
    return nc


def kernel(**inputs):
    global _PROG, _PROG_KEY
    from concourse.bass_utils import run_bass_kernel_spmd
    in_maps, order, TA, TB = _host_prep(inputs)
    zb = not np.any(inputs["qkv_b"][:, 2 * D:])
    if _PROG is None or _PROG_KEY != (TA, TB, zb):
        _PROG = _build_bass(TA, TB, zb)
        _PROG_KEY = (TA, TB, zb)
    res = run_bass_kernel_spmd(_PROG, in_maps, list(range(NCORES)))
    out = np.zeros((B, M, VAE), np.float32)
    for c in range(NCORES):
        o = res.results[c]["out"]
        out[int(order[c])] = o[0]
        out[int(order[NCORES + c])] = o[1]
    return out.astype(np.float32)
